# revision 1
# baseline (speedup 1.0000x reference)
"""Two-layer GAT (KeypointGraph) on 8 Trainium2 NeuronCores.

Strategy (dst-sharded message passing, window-batched, 3-way split-H overlap):
 - Host: add self-loops, partition edges by destination node into 8 cores x
   1088 dst nodes x 9 windows of 128 dsts; within each window edges are split
   by src range into LO (<2944) / MID (<5888) / HI tile groups, each padded to
   128-edge tiles; per-tile one-hot matrices med/mde packed per window (bf16).
 - Device (one NEFF, run once per GAT layer, SPMD on 8 cores):
   H: per block aux matmul X_b @ [W@a_src | W@a_dst] into a PSUM strip (all 69
     done by ~8us; adst extracted + written to ADSTT via the idle Pool queue),
     then the 1024-col feature matmuls; rows [h|asrc] written to HTAB_L (blocks
     0-22) / HTAB_M (23-45) / HTAB_H (46-68) so gathers overlap most of H.
   Phase E per window: per tile indirect row gather into two group-aligned
     window tiles (gwA = LO+MID tiles, triple-buffered; gwB = HI); psa matmul
     (mde^T @ adst_win) into a PSUM strip; batched logits (strided add per
     half + Prelu(0.2) + Exp) -> exwf f32 + exw bf16; per tile scale the med
     one-hot by exw per head (DVE h0,h1,h2; Act h3 and h2 every 4th tile) and
     run 4 accumulating po_h matmuls (own PSUM banks) + den; epilogue
     rec=0.25/den, per-head Act scale, adds + bias -> Y.
 - Host between layers: x2 = relu(y1), transpose/cast -> rerun same NEFF with
   layer-2 weights.
"""

import sys

sys.path.insert(0, "/opt/trn_rl_repo")

import numpy as np
import ml_dtypes

import concourse.bass as bass
import concourse.mybir as mybir
import concourse.tile as tile
from concourse.bass import ts
from concourse.bass_utils import run_bass_kernel_spmd

BF16 = ml_dtypes.bfloat16

B, K, F = 512, 17, 256
N = B * K              # 8704
HEADS, C = 4, 256
HC = HEADS * C         # 1024
NAUG = HC + 8          # 1032
NCORES = 8
NPC = N // NCORES      # 1088 dst nodes per core
NWIN = 9               # 8 full 128-dst windows + 1 half window
NPAD = 8832            # node table rows (8704 real + pad row 8704 + align)
PADROW = N             # gather index for padding edges
NB = NPAD // 128       # 69 H blocks
ROWW = HC + 4          # 1028 table row width
NBL = 15               # LO table blocks (written first; gathers overlap H)
NBM = 23               # MID table blocks
SPL = NBL * 128        # 2944
SPM = (NBL + NBM) * 128  # 5888

_cache = {}


def _split_multiwaits(nc):
    """This image's walrus supports only ONE sync-wait command per
    instruction; hoist extra waits onto prepended same-engine NoOps."""
    for f in nc.m.functions:
        for blk in f.blocks:
            old = blk.instructions
            new = []
            changed = False
            for inst in old:
                si = inst.sync_info
                if si is not None and len(si.on_wait) > 1:
                    waits = list(si.on_wait)
                    for k, w in enumerate(waits[:-1]):
                        new.append(
                            mybir.InstNoOp(
                                name=f"{inst.name}_wsplit{k}",
                                engine=inst.engine,
                                sync_info=mybir.SyncInfo(on_wait=[w], on_update=[]),
                                bass_nofuse=True,
                            )
                        )
                    inst.sync_info = mybir.SyncInfo(
                        on_wait=[waits[-1]], on_update=list(si.on_update)
                    )
                    changed = True
                new.append(inst)
            if changed:
                blk.instructions = new


def _build_layer_nc(tw, twl, twm):
    """One GAT layer, SPMD over 8 cores.

    tw[w]: total tiles; twl[w]/twm[w]: LO/MID tile counts per window."""
    nc = bass.Bass(num_devices=NCORES)
    dt = mybir.dt
    twmax = max(tw)
    twab = [twl[w] + twm[w] for w in range(NWIN)]   # gwA tiles per window
    TA = max(twab)
    TBH = max(tw[w] - twab[w] for w in range(NWIN))

    XT = nc.dram_tensor("xt", [2, 128, NPAD], dt.bfloat16, kind="ExternalInput")
    WAUG = nc.dram_tensor("waug", [2, 128, NAUG], dt.bfloat16, kind="ExternalInput")
    BIAS = nc.dram_tensor("bias", [128, C], dt.float32, kind="ExternalInput")
    SRC = nc.dram_tensor("src", [128, NWIN * twmax], dt.int32, kind="ExternalInput")
    ADIX = nc.dram_tensor("adix", [128, NWIN], dt.int32, kind="ExternalInput")
    MEDE = nc.dram_tensor(
        "mede", [NWIN, 128, twmax * 256], dt.bfloat16, kind="ExternalInput"
    )
    Y = nc.dram_tensor("y", [NWIN, 128, C], dt.float32, kind="ExternalOutput")

    HTL = nc.dram_tensor("htl", [SPL, ROWW], dt.bfloat16)
    HTM = nc.dram_tensor("htm", [SPM - SPL, ROWW], dt.bfloat16)
    HTH = nc.dram_tensor("hth", [NPAD - SPM, ROWW], dt.bfloat16)
    ADSTT = nc.dram_tensor("adstt", [NPAD, 4], dt.float32)

    with tile.TileContext(nc) as tc:
        with (
            tc.tile_pool(name="per", bufs=1) as per,
            tc.tile_pool(name="gwa", bufs=5) as gwa,
            tc.tile_pool(name="mw", bufs=3) as mw,
            tc.tile_pool(name="sm", bufs=2) as sm,
            tc.tile_pool(name="mx", bufs=8) as mxp,
            tc.tile_pool(name="yt", bufs=2) as yt,
            tc.tile_pool(name="ppo", bufs=1, space="PSUM") as ppo,
            tc.tile_pool(name="pua", bufs=1, space="PSUM") as pua,
            tc.tile_pool(name="pax", bufs=2, space="PSUM") as pax,
            tc.tile_pool(name="psw", bufs=1, space="PSUM") as pswp,
        ):
            # ---- resident inputs; xt thirds split across SP/Act queues ----
            wgs = []
            for k in range(2):
                w = per.tile([128, NAUG], dt.bfloat16, tag=f"wg{k}", name=f"wg{k}")
                (nc.sync if k == 0 else nc.scalar).dma_start(w[:], WAUG[k])
                wgs.append(w)
            bia = per.tile([128, C], dt.float32, tag="bias")
            nc.scalar.dma_start(bia[:], BIAS[:])
            xtp_cm = tc.tile_pool(name="xtp", bufs=1)
            xtp = xtp_cm.__enter__()
            hp_cm = tc.tile_pool(name="hsb", bufs=8)
            hpool = hp_cm.__enter__()
            xts = []
            for k in range(2):
                x = xtp.tile([128, NPAD], dt.bfloat16, tag=f"xt{k}", name=f"xtt{k}")
                xts.append(x)
            for c0, c1 in ((0, SPL), (SPL, SPM), (SPM, NPAD)):
                for k in range(2):
                    eng = nc.sync if k == 0 else nc.scalar
                    eng.dma_start(xts[k][:, c0:c1], XT[k, :, c0:c1])

            # ---- Pool prologue: index/medw loads before gathers ----
            medws = []
            aidxm = sm.tile([128, NWIN], dt.int32, tag="aidxm", name="aidxm")
            nc.gpsimd.dma_start(aidxm[:], ADIX[:, :])
            srcm = sm.tile([128, NWIN * twmax], dt.int32, tag="srcm", name="srcm")
            nc.gpsimd.dma_start(srcm[:], SRC[:, :])
            for w in range(3):
                medw = mw.tile([128, twmax * 256], dt.bfloat16, tag="medw",
                               name=f"medw{w}")
                nc.gpsimd.dma_start(medw[:, 0 : tw[w] * 256], MEDE[w, :, 0 : tw[w] * 256])
                medws.append(medw)

            # ---- H: aux matmuls + feature blocks; LO/MID/HI tables ----
            NBA = 64
            auxA = pua.tile([128, 8 * NBA], dt.float32, tag="auxA")
            auxB = pax.tile([128, 40], dt.float32, tag="aux8", name="auxB")

            def aux_slice(nb, n=8):
                if nb < NBA:
                    return auxA[:, 8 * nb : 8 * nb + n]
                return auxB[:, 8 * (nb - NBA) : 8 * (nb - NBA) + n]

            def emit_aux(nb):
                for k in range(2):
                    nc.tensor.matmul(
                        aux_slice(nb),
                        lhsT=xts[k][:, ts(nb, 128)],
                        rhs=wgs[k][:, 1024:1032],
                        start=(k == 0),
                        stop=(k == 1),
                    )

            def emit_block(nb):
                hsb = hpool.tile([128, ROWW], dt.bfloat16, tag="hsb",
                                 name=f"hsb{nb}")
                for ci, c0 in enumerate((0, 512)):
                    slot = (2 * nb + ci) % 6
                    pool_, tag_ = (
                        (ppo, f"po{slot}") if slot < 4
                        else ((pswp, "psw") if slot == 4 else (pax, "aux8"))
                    )
                    ps = pool_.tile(
                        [128, 512], dt.float32, name=f"hps{nb}_{ci}", tag=tag_,
                    )
                    for k in range(2):
                        nc.tensor.matmul(
                            ps[:],
                            lhsT=xts[k][:, ts(nb, 128)],
                            rhs=wgs[k][:, c0 : c0 + 512],
                            start=(k == 0),
                            stop=(k == 1),
                        )
                    if ci == 0:
                        nc.scalar.copy(hsb[:, 0:512], ps[:])
                    else:
                        nc.vector.tensor_copy(hsb[:, 512:1024], ps[:])
                if nb % 2 == 0:
                    nc.vector.tensor_copy(hsb[:, 1024:1028], aux_slice(nb, 4))
                else:
                    nc.scalar.copy(hsb[:, 1024:1028], aux_slice(nb, 4))
                if nb < NBL:
                    nc.sync.dma_start(HTL[ts(nb, 128), :], hsb[:])
                elif nb < NBL + NBM:
                    nc.sync.dma_start(HTM[ts(nb - NBL, 128), :], hsb[:])
                else:
                    nc.sync.dma_start(HTH[ts(nb - NBL - NBM, 128), :], hsb[:])

            for nb in range(NBL):
                emit_aux(nb)
            for nb in range(0, 5):
                emit_block(nb)
            for nb in range(NBL, NB):
                emit_aux(nb)
            asb = per.tile([128, 4 * NB], dt.float32, tag="asb")
            nc.vector.tensor_copy(
                asb[:, 0 : 4 * NBA].rearrange("p (b c) -> p b c", b=NBA, c=4),
                auxA[:].rearrange("p (b c) -> p b c", b=NBA, c=8)[:, :, 4:8],
            )
            nc.vector.tensor_copy(
                asb[:, 4 * NBA : 4 * NB].rearrange("p (b c) -> p b c", b=NB - NBA, c=4),
                auxB[:].rearrange("p (b c) -> p b c", b=NB - NBA, c=8)[:, :, 4:8],
            )
            nc.gpsimd.dma_start(
                ADSTT[:, :].rearrange("(b p) c -> p b c", b=NB, p=128),
                asb[:].rearrange("p (b c) -> p b c", b=NB, c=4),
            )
            adwbs = []
            for w in range(NWIN):
                adw = sm.tile([128, 4], dt.float32, tag="adw", bufs=NWIN,
                              name=f"adw{w}")
                nc.gpsimd.indirect_dma_start(
                    out=adw[:],
                    out_offset=None,
                    in_=ADSTT[:, :],
                    in_offset=bass.IndirectOffsetOnAxis(
                        ap=aidxm[:, w : w + 1], axis=0
                    ),
                )
                adwb = sm.tile([128, 4], dt.bfloat16, tag="adwb", bufs=NWIN,
                               name=f"adwb{w}")
                nc.vector.tensor_copy(adwb[:], adw[:])
                adwbs.append(adwb)
            for nb in range(5, NB):
                emit_block(nb)
            hp_cm.__exit__(None, None, None)
            xtp_cm.__exit__(None, None, None)
            gwb_cm = tc.tile_pool(name="gwb", bufs=3)
            gwb = gwb_cm.__enter__()

            # ---- Phase E: per-window edge aggregation ----
            for w in range(NWIN):
                twn = tw[w]
                na = twab[w]
                nb_ = twn - na
                if w >= 3:
                    medw = mw.tile([128, twmax * 256], dt.bfloat16, tag="medw",
                                   name=f"medw{w}")
                    nc.sync.dma_start(
                        medw[:, 0 : twn * 256], MEDE[w, :, 0 : twn * 256]
                    )
                    medws.append(medw)
                medw = medws[w]
                sidxw = srcm[:, w * twmax : (w + 1) * twmax]

                gwA = gwa.tile([128, TA * ROWW], dt.bfloat16, tag="gwA",
                               name=f"gwA{w}")
                gwB = gwb.tile([128, TBH * ROWW], dt.bfloat16, tag="gwB",
                               name=f"gwB{w}")

                def gslice(t, c0, c1, na=na, gwA=gwA, gwB=gwB):
                    if t < na:
                        return gwA[:, t * ROWW + c0 : t * ROWW + c1]
                    tb = t - na
                    return gwB[:, tb * ROWW + c0 : tb * ROWW + c1]

                psw = pswp.tile([128, 4 * twmax], dt.float32, tag="psw",
                                name=f"psw{w}")

                for t in range(twn):
                    if t < twl[w]:
                        htab = HTL
                    elif t < twab[w]:
                        htab = HTM
                    else:
                        htab = HTH
                    nc.gpsimd.indirect_dma_start(
                        out=gslice(t, 0, ROWW),
                        out_offset=None,
                        in_=htab[:, :],
                        in_offset=bass.IndirectOffsetOnAxis(
                            ap=sidxw[:, t : t + 1], axis=0
                        ),
                    )

                adwb = adwbs[w]
                for t in range(twn):
                    nc.tensor.matmul(
                        psw[:, 4 * t : 4 * t + 4],
                        lhsT=medw[:, 256 * t + 128 : 256 * t + 256],
                        rhs=adwb[:],
                        start=True,
                        stop=True,
                    )

                # batched logits, one strided add per gather half
                eff = sm.tile([128, 4 * twmax], dt.float32, tag="eff", bufs=3,
                              name=f"eff{w}")
                gvA = gwA[:, 0 : na * ROWW].rearrange(
                    "p (t c) -> p t c", t=na, c=ROWW
                )[:, :, HC : HC + 4]
                nc.vector.tensor_add(
                    eff[:, 0 : 4 * na].rearrange("p (t c) -> p t c", t=na, c=4),
                    gvA,
                    psw[:, 0 : 4 * na].rearrange("p (t c) -> p t c", t=na, c=4),
                )
                if nb_ > 0:
                    gvB = gwB[:, 0 : nb_ * ROWW].rearrange(
                        "p (t c) -> p t c", t=nb_, c=ROWW
                    )[:, :, HC : HC + 4]
                    nc.vector.tensor_add(
                        eff[:, 4 * na : 4 * twn].rearrange(
                            "p (t c) -> p t c", t=nb_, c=4
                        ),
                        gvB,
                        psw[:, 4 * na : 4 * twn].rearrange(
                            "p (t c) -> p t c", t=nb_, c=4
                        ),
                    )
                efl = sm.tile([128, 4 * twmax], dt.float32, tag="efl", bufs=3,
                              name=f"efl{w}")
                exwf = sm.tile([128, 4 * twmax], dt.float32, tag="exwf", bufs=3,
                               name=f"exwf{w}")
                exw = sm.tile([128, 4 * twmax], dt.bfloat16, tag="exw", bufs=3,
                              name=f"exw{w}")
                for lo_, hi_ in ((0, 4 * na), (4 * na, 4 * twn)):
                    if hi_ <= lo_:
                        continue
                    nc.scalar.activation(
                        efl[:, lo_:hi_], eff[:, lo_:hi_],
                        mybir.ActivationFunctionType.Prelu, alpha=0.2,
                    )
                    nc.scalar.activation(
                        exwf[:, lo_:hi_], efl[:, lo_:hi_],
                        mybir.ActivationFunctionType.Exp,
                    )
                    nc.vector.tensor_copy(exw[:, lo_:hi_], exwf[:, lo_:hi_])

                pos = [
                    ppo.tile([128, 512], dt.float32, name=f"po_{w}_{h}", tag=f"po{h}")
                    for h in range(4)
                ]
                den = pax.tile([128, 40], dt.float32, tag="aux8", name=f"den{w}")

                for t in range(twn):
                    first = t == 0
                    last = t == twn - 1
                    mx = mxp.tile([128, 512], dt.bfloat16, tag="mx",
                                  name=f"mx_{w}_{t}")
                    if t % 3 != 2:
                        # DVE: one fused 4-head broadcast multiply
                        nc.vector.tensor_mul(
                            mx[:].rearrange("p (h c) -> p h c", h=4, c=128),
                            medw[:, 256 * t : 256 * t + 128].unsqueeze(1)
                                .to_broadcast([128, 4, 128]),
                            exw[:, 4 * t : 4 * t + 4].unsqueeze(2)
                                .to_broadcast([128, 4, 128]),
                        )
                    else:
                        # Act: per-head scalar-scale copies
                        for h in range(HEADS):
                            nc.scalar.mul(
                                mx[:, 128 * h : 128 * (h + 1)],
                                medw[:, 256 * t : 256 * t + 128],
                                exwf[:, 4 * t + h : 4 * t + h + 1],
                            )
                    for h in range(HEADS):
                        nc.tensor.matmul(
                            pos[h][:, 0:C],
                            lhsT=mx[:, 128 * h : 128 * (h + 1)],
                            rhs=gslice(t, h * C, (h + 1) * C),
                            start=first,
                            stop=last,
                        )
                    nc.tensor.matmul(
                        den[:, 0:4],
                        lhsT=medw[:, 256 * t : 256 * t + 128],
                        rhs=exw[:, 4 * t : 4 * t + 4],
                        start=first,
                        stop=last,
                    )

                rec = sm.tile([128, 4], dt.float32, tag="rec", name=f"rec{w}")
                nc.vector.reciprocal(rec[:], den[:, 0:4])
                recq = sm.tile([128, 4], dt.float32, tag="recq", name=f"recq{w}")
                nc.scalar.mul(recq[:], rec[:], 1.0 / HEADS)
                # two fused mul-add chains in parallel: DVE heads 0,1; Act 2,3
                a1 = yt.tile([128, C], dt.float32, tag="yh0", name=f"a1_{w}")
                nc.vector.scalar_tensor_tensor(
                    a1[:], pos[1][:, 0:C], recq[:, 1:2], bia[:],
                    mybir.AluOpType.mult, mybir.AluOpType.add,
                )
                a2 = yt.tile([128, C], dt.float32, tag="yh1", name=f"a2_{w}")
                nc.vector.scalar_tensor_tensor(
                    a2[:], pos[0][:, 0:C], recq[:, 0:1], a1[:],
                    mybir.AluOpType.mult, mybir.AluOpType.add,
                )
                b1 = yt.tile([128, C], dt.float32, tag="yh2", name=f"b1_{w}")
                nc.scalar.mul(b1[:], pos[3][:, 0:C], recq[:, 3:4])
                b2 = yt.tile([128, C], dt.float32, tag="yh3", name=f"b2_{w}")
                nc.vector.scalar_tensor_tensor(
                    b2[:], pos[2][:, 0:C], recq[:, 2:3], b1[:],
                    mybir.AluOpType.mult, mybir.AluOpType.add,
                )
                yacc = yt.tile([128, C], dt.float32, tag="yacc", name=f"yacc{w}")
                nc.vector.tensor_add(yacc[:], a2[:], b2[:])
                nc.sync.dma_start(Y[w], yacc[:])
            gwb_cm.__exit__(None, None, None)

    _split_multiwaits(nc)
    return nc


def _host_prep(edge_index):
    ei = np.asarray(edge_index).astype(np.int64)
    loop = np.arange(N, dtype=np.int64)
    src = np.concatenate([ei[0], loop])
    dst = np.concatenate([ei[1], loop])
    grp = (src >= SPL).astype(np.int64) + (src >= SPM)

    # ---- balance dsts into (core, window) buckets so every bucket has
    # near-equal LO/MID/HI incoming-edge counts (kills ceil-padding) ----
    NBK = NCORES * NWIN
    deg = np.zeros((N, 3), np.int64)
    np.add.at(deg, (dst, grp), 1)
    cap = np.where(np.arange(NBK) % NWIN == NWIN - 1, 64, 128)
    targ = deg.sum(0).astype(np.float64) / (N / 128.0)  # per full bucket
    targ_b = targ[None, :] * (cap[:, None] / 128.0)
    order = np.argsort(-deg.sum(1), kind="stable")
    L = np.zeros((NBK, 3), np.float64)
    nfill = np.zeros(NBK, np.int64)
    pj = np.zeros(N, np.int64)
    pw = np.zeros(N, np.int64)
    pslot = np.zeros(N, np.int64)
    for d in order:
        over = (L + deg[d][None, :]) - targ_b
        score = over.max(1)
        score[nfill >= cap] = np.inf
        b = int(np.argmin(score))
        L[b] += deg[d]
        pj[d] = b // NWIN
        pw[d] = b % NWIN
        pslot[d] = nfill[b]
        nfill[b] += 1

    # per (core, window) edge lists, split by src table group
    ecore = pj[dst]
    ewin = pw[dst]
    dstw = pslot[dst]

    cnt = np.zeros((3, NCORES, NWIN), np.int64)
    for j in range(NCORES):
        m = ecore == j
        for w in range(NWIN):
            mw_ = m & (ewin == w)
            for g in range(3):
                cnt[g, j, w] = int((mw_ & (grp == g)).sum())
    twl = [int(np.ceil(cnt[0, :, w].max() / 128)) for w in range(NWIN)]
    twm = [int(np.ceil(cnt[1, :, w].max() / 128)) for w in range(NWIN)]
    twh = [int(np.ceil(cnt[2, :, w].max() / 128)) for w in range(NWIN)]
    tw = [twl[w] + twm[w] + twh[w] for w in range(NWIN)]
    T = sum(tw)
    twmax = max(tw)
    gbase = np.array([0, SPL, SPM])

    srcw = np.zeros((NCORES, NWIN, 128, twmax), np.int32)
    dstwin = np.full((NCORES, NWIN, 128, twmax), -1, np.int64)
    for j in range(NCORES):
        m = ecore == j
        for w in range(NWIN):
            mw_ = m & (ewin == w)
            t0 = 0
            for g, gtw in ((0, twl[w]), (1, twm[w]), (2, twh[w])):
                sel = mw_ & (grp == g)
                s = src[sel] - gbase[g]
                d = dstw[sel]
                cntg = len(s)
                es = np.arange(cntg)
                srcw[j, w, es % 128, t0 + es // 128] = s.astype(np.int32)
                dstwin[j, w, es % 128, t0 + es // 128] = d
                t0 += gtw

    iota = np.arange(128)
    med = (dstwin[..., None] == iota[None, None, None, None, :]).astype(BF16)
    mde = med.transpose(0, 1, 4, 3, 2).copy()
    mede = np.empty((NCORES, NWIN, 128, twmax, 256), BF16)
    mede[..., 0:128] = med
    mede[..., 128:256] = mde
    mede = mede.reshape(NCORES, NWIN, 128, twmax * 256).copy()

    # per-slot adst row ids (global node ids; pad slots -> last table row)
    adix4 = np.full((NCORES, NWIN, 128), NPAD - 1, np.int32)
    adix4[pj, pw, pslot] = np.arange(N, dtype=np.int32)
    adix = adix4.transpose(0, 2, 1).copy()            # [NC, 128, NWIN]
    srcw = srcw.transpose(0, 2, 1, 3).reshape(NCORES, 128, NWIN * twmax).copy()
    return tw, twl, twm, T, srcw, mede, adix, (pj, pw, pslot)


def _aug_weights(W, a_src, a_dst):
    W64 = np.asarray(W, np.float64)
    As = np.asarray(a_src, np.float64)
    Ad = np.asarray(a_dst, np.float64)
    Wh = W64.reshape(W64.shape[0], HEADS, C)
    wa_s = (Wh * As[None]).sum(-1)  # [K, HEADS]
    wa_d = (Wh * Ad[None]).sum(-1)
    waug = np.concatenate([W64, wa_s, wa_d], axis=1)  # [K, 1032]
    return waug.astype(BF16).reshape(2, 128, NAUG)


def _xt_pad(x):
    """x [N, 256] f32 -> XT bf16 [2, 128, NPAD] (zero-padded cols)."""
    xt = np.zeros((256, NPAD), np.float32)
    xt[:, :N] = np.asarray(x, np.float32).T
    return xt.astype(BF16).reshape(2, 128, NPAD)


def _run_layer(nc, xt, waug, bias, srcw, mede, adix, placement):
    bias_b = np.broadcast_to(np.asarray(bias, np.float32)[None, :], (128, C)).copy()
    in_maps = []
    for j in range(NCORES):
        in_maps.append(
            {
                "xt": xt,
                "waug": waug,
                "bias": bias_b,
                "src": srcw[j],
                "adix": adix[j],
                "mede": mede[j],
            }
        )
    res = run_bass_kernel_spmd(nc, in_maps, core_ids=list(range(NCORES)))
    pj, pw, pslot = placement
    yall = np.stack([res.results[j]["y"] for j in range(NCORES)])  # [NC,NWIN,128,C]
    y = yall[pj, pw, pslot].astype(np.float32)
    return y, res


def kernel(kpt_feature, edge_index, W1, a_src1, a_dst1, b1, W2, a_src2, a_dst2, b2):
    key = "k"
    if key not in _cache:
        tw, twl, twm, T, srcw, mede, adix, placement = _host_prep(edge_index)
        nc = _build_layer_nc(tw, twl, twm)
        _cache[key] = (nc, tw, T, srcw, mede, adix, placement)
    nc, tw, T, srcw, mede, adix, placement = _cache[key]

    x1 = np.asarray(kpt_feature, np.float32).reshape(N, F)
    y1, _ = _run_layer(
        nc, _xt_pad(x1), _aug_weights(W1, a_src1, a_dst1), b1, srcw, mede, adix,
        placement,
    )
    x2 = np.maximum(y1, 0.0)
    y2, _ = _run_layer(
        nc, _xt_pad(x2), _aug_weights(W2, a_src2, a_dst2), b2, srcw, mede, adix,
        placement,
    )
    return y2.reshape(B, K, F).astype(np.float32)



# revision 4
# speedup vs baseline: 1.4562x; 1.4562x over previous
"""Two-layer GAT (KeypointGraph) on 8 Trainium2 NeuronCores.

Strategy (x-space aggregation, dst-sharded, window-pipelined):
 - GAT algebra: out[d] = (1/4)·Σ_h (Σ_e α_eh x[src_e]) @ W_h + b — the linear
   transform commutes with the α-weighted aggregation, so each core aggregates
   256-wide x rows (not 1024-wide h rows) and applies W once per 128-dst
   window. No replicated X@W, no H table round trip; gather traffic drops 4x.
 - Host: add self-loops, balance dst nodes into 8 cores x 9 windows of 128
   dst slots (window 8 half-filled); per (core,window) edges padded to 128-edge
   tiles; one-hot med/mde per tile (bf16); gather table TAB[n,260] = [x | x@Wa_src]
   (bf16); per-window dst attention ADW = x@Wa_dst permuted to slots.
 - Device (one NEFF per layer run, SPMD on 8 cores), per window:
   per-tile indirect row gather (Pool) -> xg; psa matmul (mde^T @ adw) gives
   per-edge a_dst; logits = gathered a_src + psa -> Prelu -> Exp (Act);
   den[d,h] via med^T @ exw (PE, ~free); rec = 1/den; per-edge rec via
   mde^T @ rec; exwn = exw·rec (normalized weights); mx = med x exwn broadcast
   (DVE/Act split); two accumulating zT matmuls per tile:
   zT[c,(h,d)] += xg[:,chalf]^T @ mx  (PSUM, 2 banks); after the window:
   zT -> SBUF bf16, 8 accumulating matmuls vs W chunks -> out[d,256] (+bias).
 - Host between layers: x2 = relu(y1); rebuild TAB/ADW with layer-2 weights and
   rerun the same NEFF.
"""

import sys

sys.path.insert(0, "/opt/trn_rl_repo")

import numpy as np
import ml_dtypes

import concourse.bass as bass
import concourse.mybir as mybir
import concourse.tile as tile
from concourse.bass_utils import run_bass_kernel_spmd

BF16 = ml_dtypes.bfloat16

B, K, F = 512, 17, 256
N = B * K              # 8704
HEADS, C = 4, 256
NCORES = 8
NPC = N // NCORES      # 1088 dst nodes per core
NWIN = 9               # 8 full 128-dst windows + 1 half window
NPAD = 8832            # gather table rows (8704 real + pad)
PADROW = N             # gather index for padding edges
ROWW = 260             # table row: 256 x cols + 4 a_src cols (520B)
NEG_SLOPE = 0.2

_cache = {}


def _split_multiwaits(nc):
    """This image's walrus supports only ONE sync-wait command per
    instruction; hoist extra waits onto prepended same-engine NoOps."""
    for f in nc.m.functions:
        for blk in f.blocks:
            old = blk.instructions
            new = []
            changed = False
            for inst in old:
                si = inst.sync_info
                if si is not None and len(si.on_wait) > 1:
                    waits = list(si.on_wait)
                    for k, w in enumerate(waits[:-1]):
                        new.append(
                            mybir.InstNoOp(
                                name=f"{inst.name}_wsplit{k}",
                                engine=inst.engine,
                                sync_info=mybir.SyncInfo(on_wait=[w], on_update=[]),
                                bass_nofuse=True,
                            )
                        )
                    inst.sync_info = mybir.SyncInfo(
                        on_wait=[waits[-1]], on_update=list(si.on_update)
                    )
                    changed = True
                new.append(inst)
            if changed:
                blk.instructions = new


def _build_layer_nc(tw, mx_sched):
    """One GAT layer, SPMD over 8 cores. tw[w]: tiles per window."""
    nc = bass.Bass(num_devices=NCORES)
    dt = mybir.dt
    twmax = max(tw)

    TAB = nc.dram_tensor("tab", [NPAD, ROWW], dt.bfloat16, kind="ExternalInput")
    WG = nc.dram_tensor("wg", [2, 128, HEADS * C], dt.bfloat16, kind="ExternalInput")
    BIAS = nc.dram_tensor("bias", [128, C], dt.float32, kind="ExternalInput")
    SRC = nc.dram_tensor("src", [128, NWIN * twmax], dt.int32, kind="ExternalInput")
    ADW = nc.dram_tensor("adw", [128, NWIN * 4], dt.bfloat16, kind="ExternalInput")
    MEDE = nc.dram_tensor(
        "mede", [NWIN, 128, twmax * 256], dt.bfloat16, kind="ExternalInput"
    )
    Y = nc.dram_tensor("y", [NWIN, 128, C], dt.float32, kind="ExternalOutput")

    with tile.TileContext(nc) as tc:
        with (
            tc.tile_pool(name="per", bufs=1) as per,
            tc.tile_pool(name="mw", bufs=3) as mw,
            tc.tile_pool(name="xg", bufs=3) as xgp,
            tc.tile_pool(name="sm", bufs=3) as sm,
            tc.tile_pool(name="mx", bufs=8) as mxp,
            tc.tile_pool(name="zs", bufs=2) as zs,
            tc.tile_pool(name="yt", bufs=2) as yt,
            tc.tile_pool(name="ppz", bufs=2, space="PSUM") as ppz,
            tc.tile_pool(name="pst", bufs=2, space="PSUM") as pst,
            tc.tile_pool(name="pot", bufs=2, space="PSUM") as pot,
        ):
            # ---- resident inputs ----
            wgs = []
            for k in range(2):
                w_ = per.tile([128, HEADS * C], dt.bfloat16, tag=f"wg{k}",
                              name=f"wg{k}")
                nc.scalar.dma_start(w_[:], WG[k])
                wgs.append(w_)
            bia = per.tile([128, C], dt.float32, tag="bias")
            nc.scalar.dma_start(bia[:], BIAS[:])
            srcm = per.tile([128, NWIN * twmax], dt.int32, tag="srcm", name="srcm")
            nc.sync.dma_start(srcm[:], SRC[:, :])
            adwsb = per.tile([128, NWIN * 4], dt.bfloat16, tag="adw", name="adwsb")
            nc.sync.dma_start(adwsb[:], ADW[:, :])

            for w in range(NWIN):
                twn = tw[w]
                medw = mw.tile([128, twmax * 256], dt.bfloat16, tag="medw",
                               name=f"medw{w}")
                nc.sync.dma_start(medw[:, 0 : twn * 256], MEDE[w, :, 0 : twn * 256])

                # ---- per-tile indirect row gathers (Pool) ----
                xgw = xgp.tile([128, twmax * ROWW], dt.bfloat16, tag="xgw",
                               name=f"xgw{w}")
                sidxw = srcm[:, w * twmax : w * twmax + twn]
                for t in range(twn):
                    nc.gpsimd.indirect_dma_start(
                        out=xgw[:, t * ROWW : (t + 1) * ROWW],
                        out_offset=None,
                        in_=TAB[:, :],
                        in_offset=bass.IndirectOffsetOnAxis(
                            ap=sidxw[:, t : t + 1], axis=0
                        ),
                    )

                # ---- strips: psa (per-edge a_dst), den, recp share one bank ----
                psT = pst.tile([128, 512], dt.float32, tag="pst", name=f"pst{w}")
                for t in range(twn):
                    nc.tensor.matmul(
                        psT[:, 4 * t : 4 * t + 4],
                        lhsT=medw[:, 256 * t + 128 : 256 * t + 256],
                        rhs=adwsb[:, 4 * w : 4 * w + 4],
                        start=True,
                        stop=True,
                    )

                # logits: eff = a_src(gathered) + psa; Prelu; Exp
                eff = sm.tile([128, 4 * twmax], dt.float32, tag="eff",
                              name=f"eff{w}")
                gv = xgw[:, 0 : twn * ROWW].rearrange(
                    "p (t c) -> p t c", t=twn, c=ROWW
                )[:, :, 256:260]
                nc.vector.tensor_add(
                    eff[:, 0 : 4 * twn].rearrange("p (t c) -> p t c", t=twn, c=4),
                    gv,
                    psT[:, 0 : 4 * twn].rearrange("p (t c) -> p t c", t=twn, c=4),
                )
                efl = sm.tile([128, 4 * twmax], dt.float32, tag="efl",
                              name=f"efl{w}")
                nc.scalar.activation(
                    efl[:, 0 : 4 * twn], eff[:, 0 : 4 * twn],
                    mybir.ActivationFunctionType.Prelu, alpha=NEG_SLOPE,
                )
                exwf = sm.tile([128, 4 * twmax], dt.float32, tag="exwf",
                               name=f"exwf{w}")
                nc.scalar.activation(
                    exwf[:, 0 : 4 * twn], efl[:, 0 : 4 * twn],
                    mybir.ActivationFunctionType.Exp,
                )
                exwb = sm.tile([128, 4 * twmax], dt.bfloat16, tag="exwb",
                               name=f"exwb{w}")
                nc.vector.tensor_copy(exwb[:, 0 : 4 * twn], exwf[:, 0 : 4 * twn])

                # den[d,h] = sum_e exw; rec = 1/den (bf16); recp[e,h] = rec[dst_e]
                for t in range(twn):
                    nc.tensor.matmul(
                        psT[:, 96:100],
                        lhsT=medw[:, 256 * t : 256 * t + 128],
                        rhs=exwb[:, 4 * t : 4 * t + 4],
                        start=(t == 0),
                        stop=(t == twn - 1),
                    )
                recf = sm.tile([128, 4], dt.float32, tag="recf", name=f"recf{w}")
                nc.vector.reciprocal(recf[:], psT[:, 96:100])
                recb = sm.tile([128, 4], dt.bfloat16, tag="recb", name=f"recb{w}")
                nc.vector.tensor_copy(recb[:], recf[:])
                for t in range(twn):
                    nc.tensor.matmul(
                        psT[:, 128 + 4 * t : 128 + 4 * t + 4],
                        lhsT=medw[:, 256 * t + 128 : 256 * t + 256],
                        rhs=recb[:],
                        start=True,
                        stop=True,
                    )
                exwnf = sm.tile([128, 4 * twmax], dt.float32, tag="exwnf",
                                name=f"exwnf{w}")
                nc.vector.tensor_mul(
                    exwnf[:, 0 : 4 * twn], exwf[:, 0 : 4 * twn],
                    psT[:, 128 : 128 + 4 * twn],
                )
                exwn = sm.tile([128, 4 * twmax], dt.bfloat16, tag="exwn",
                               name=f"exwn{w}")
                nc.vector.tensor_copy(exwn[:, 0 : 4 * twn], exwnf[:, 0 : 4 * twn])

                # ---- weighted aggregation in x-space ----
                ztA = ppz.tile([128, 512], dt.float32, tag="ztA", name=f"ztA{w}")
                ztB = ppz.tile([128, 512], dt.float32, tag="ztB", name=f"ztB{w}")
                for t in range(twn):
                    first = t == 0
                    last = t == twn - 1
                    mx = mxp.tile([128, 512], dt.bfloat16, tag="mx",
                                  name=f"mx_{w}_{t}")
                    if mx_sched[w][t] == 0:
                        # DVE: one fused 4-head broadcast multiply
                        nc.vector.tensor_mul(
                            mx[:].rearrange("p (h c) -> p h c", h=4, c=128),
                            medw[:, 256 * t : 256 * t + 128].unsqueeze(1)
                                .to_broadcast([128, 4, 128]),
                            exwn[:, 4 * t : 4 * t + 4].unsqueeze(2)
                                .to_broadcast([128, 4, 128]),
                        )
                    else:
                        # Act: per-head scalar-scale copies
                        for h in range(HEADS):
                            nc.scalar.mul(
                                mx[:, 128 * h : 128 * (h + 1)],
                                medw[:, 256 * t : 256 * t + 128],
                                exwnf[:, 4 * t + h : 4 * t + h + 1],
                            )
                    nc.tensor.matmul(
                        ztA[:],
                        lhsT=xgw[:, t * ROWW : t * ROWW + 128],
                        rhs=mx[:],
                        start=first,
                        stop=last,
                    )
                    nc.tensor.matmul(
                        ztB[:],
                        lhsT=xgw[:, t * ROWW + 128 : t * ROWW + 256],
                        rhs=mx[:],
                        start=first,
                        stop=last,
                    )

                # ---- per-window transform: out = sum_h z_h @ W_h/4 + b ----
                zsa = zs.tile([128, 512], dt.bfloat16, tag="zsa", name=f"zsa{w}")
                nc.vector.tensor_copy(zsa[:], ztA[:])
                zsb = zs.tile([128, 512], dt.bfloat16, tag="zsb", name=f"zsb{w}")
                nc.scalar.copy(zsb[:], ztB[:])
                outw = pot.tile([128, C], dt.float32, tag="outw", name=f"outw{w}")
                for h in range(HEADS):
                    nc.tensor.matmul(
                        outw[:],
                        lhsT=zsa[:, 128 * h : 128 * (h + 1)],
                        rhs=wgs[0][:, C * h : C * (h + 1)],
                        start=(h == 0),
                        stop=False,
                    )
                    nc.tensor.matmul(
                        outw[:],
                        lhsT=zsb[:, 128 * h : 128 * (h + 1)],
                        rhs=wgs[1][:, C * h : C * (h + 1)],
                        start=False,
                        stop=(h == HEADS - 1),
                    )
                yacc = yt.tile([128, C], dt.float32, tag="yacc", name=f"yacc{w}")
                nc.vector.tensor_add(yacc[:], outw[:], bia[:])
                nc.sync.dma_start(Y[w], yacc[:])

    _split_multiwaits(nc)
    return nc


def _host_prep(edge_index):
    ei = np.asarray(edge_index).astype(np.int64)
    loop = np.arange(N, dtype=np.int64)
    src = np.concatenate([ei[0], loop])
    dst = np.concatenate([ei[1], loop])

    # ---- balance dsts into (core, window) buckets by total degree ----
    NBK = NCORES * NWIN
    deg = np.bincount(dst, minlength=N).astype(np.int64)
    cap = np.where(np.arange(NBK) % NWIN == NWIN - 1, 64, 128)
    targ = deg.sum() / (N / 128.0)
    targ_b = targ * (cap / 128.0)
    order = np.argsort(-deg, kind="stable")
    L = np.zeros(NBK, np.float64)
    nfill = np.zeros(NBK, np.int64)
    pj = np.zeros(N, np.int64)
    pw = np.zeros(N, np.int64)
    pslot = np.zeros(N, np.int64)
    for d in order:
        score = (L + deg[d]) - targ_b
        score[nfill >= cap] = np.inf
        b = int(np.argmin(score))
        L[b] += deg[d]
        pj[d] = b // NWIN
        pw[d] = b % NWIN
        pslot[d] = nfill[b]
        nfill[b] += 1

    ecore = pj[dst]
    ewin = pw[dst]
    dstw = pslot[dst]

    cnt = np.zeros((NCORES, NWIN), np.int64)
    for j in range(NCORES):
        m = ecore == j
        for w in range(NWIN):
            cnt[j, w] = int((m & (ewin == w)).sum())
    # pad dst slots (window 8 has 64 empty slots) get pad self-edges so that
    # den >= 1 everywhere; reserve room for them when sizing tiles
    npad_dst = np.zeros(NWIN, np.int64)
    npad_dst[NWIN - 1] = 128 - 64
    tw = [int(np.ceil((cnt[:, w].max() + npad_dst[w]) / 128)) for w in range(NWIN)]
    twmax = max(tw)
    T = sum(tw)

    srcw = np.full((NCORES, NWIN, 128, twmax), PADROW, np.int32)
    dstwin = np.full((NCORES, NWIN, 128, twmax), -1, np.int64)
    for j in range(NCORES):
        m = ecore == j
        for w in range(NWIN):
            mw_ = m & (ewin == w)
            s = src[mw_]
            d = dstw[mw_]
            cnte = len(s)
            es = np.arange(cnte)
            srcw[j, w, es % 128, es // 128] = s.astype(np.int32)
            dstwin[j, w, es % 128, es // 128] = d
            # pad self-edges for empty dst slots
            base = int(NWIN == 0)
            nreal = int(nfill[j * NWIN + w])
            pads = np.arange(nreal, 128 if w < NWIN - 1 else 128)
            pads = pads[pads < 128]
            if w == NWIN - 1:
                pads = np.arange(64, 128)
            else:
                pads = np.arange(nreal, 128)
            ep = np.arange(cnte, cnte + len(pads))
            assert ep.max(initial=-1) < twmax * 128
            srcw[j, w, ep % 128, ep // 128] = PADROW
            dstwin[j, w, ep % 128, ep // 128] = pads

    iota = np.arange(128)
    med = (dstwin[..., None] == iota[None, None, None, None, :]).astype(BF16)
    mde = med.transpose(0, 1, 4, 3, 2).copy()
    mede = np.empty((NCORES, NWIN, 128, twmax, 256), BF16)
    mede[..., 0:128] = med
    mede[..., 128:256] = mde
    mede = mede.reshape(NCORES, NWIN, 128, twmax * 256).copy()

    srcw = srcw.transpose(0, 2, 1, 3).reshape(NCORES, 128, NWIN * twmax).copy()
    return tw, T, srcw, mede, (pj, pw, pslot)


def _fold_attn(W, a):
    """Wa = W @ a per head: [F_in, HEADS]."""
    W64 = np.asarray(W, np.float64)
    A = np.asarray(a, np.float64)
    Wh = W64.reshape(W64.shape[0], HEADS, C)
    return (Wh * A[None]).sum(-1)  # [F_in, HEADS]


def _layer_inputs(x, Wl, a_src, a_dst, bias, placement):
    """Host-side per-layer tensors: TAB, ADW, WG, BIAS."""
    pj, pw, pslot = placement
    x64 = np.asarray(x, np.float64)
    asrc = x64 @ _fold_attn(Wl, a_src)          # [N, 4]
    adst = x64 @ _fold_attn(Wl, a_dst)          # [N, 4]
    tab = np.zeros((NPAD, ROWW), np.float32)
    tab[:N, 0:256] = np.asarray(x, np.float32)
    tab[:N, 256:260] = asrc
    tabb = tab.astype(BF16)
    adw = np.zeros((NCORES, NWIN, 128, 4), np.float32)
    adw[pj, pw, pslot] = adst
    adw = adw.transpose(0, 2, 1, 3).reshape(NCORES, 128, NWIN * 4).astype(BF16)
    wg = (np.asarray(Wl, np.float64) * 0.25).astype(BF16).reshape(2, 128, HEADS * C)
    bias_b = np.broadcast_to(
        np.asarray(bias, np.float32)[None, :], (128, C)
    ).copy()
    return tabb, adw, wg, bias_b


def _run_layer(nc, tabb, adw, wg, bias_b, srcw, mede, placement):
    in_maps = []
    for j in range(NCORES):
        in_maps.append(
            {
                "tab": tabb,
                "wg": wg,
                "bias": bias_b,
                "src": srcw[j],
                "adw": adw[j],
                "mede": mede[j],
            }
        )
    res = run_bass_kernel_spmd(nc, in_maps, core_ids=list(range(NCORES)))
    pj, pw, pslot = placement
    yall = np.stack([res.results[j]["y"] for j in range(NCORES)])  # [NC,NWIN,128,C]
    y = yall[pj, pw, pslot].astype(np.float32)
    return y


def _mx_schedule(tw):
    """Per (window, tile) engine for the mx broadcast: 0=DVE, 1=Act."""
    return [[0 if (t % 3) != 2 else 1 for t in range(tw[w])] for w in range(NWIN)]


def kernel(kpt_feature, edge_index, W1, a_src1, a_dst1, b1, W2, a_src2, a_dst2, b2):
    key = "k"
    if key not in _cache:
        tw, T, srcw, mede, placement = _host_prep(edge_index)
        nc = _build_layer_nc(tw, _mx_schedule(tw))
        _cache[key] = (nc, tw, T, srcw, mede, placement)
    nc, tw, T, srcw, mede, placement = _cache[key]

    x1 = np.asarray(kpt_feature, np.float32).reshape(N, F)
    y1 = _run_layer(
        nc, *_layer_inputs(x1, W1, a_src1, a_dst1, b1, placement), srcw, mede,
        placement,
    )
    x2 = np.maximum(y1, 0.0)
    y2 = _run_layer(
        nc, *_layer_inputs(x2, W2, a_src2, a_dst2, b2, placement), srcw, mede,
        placement,
    )
    return y2.reshape(B, K, F).astype(np.float32)


# revision 11
# speedup vs baseline: 1.5655x; 1.0750x over previous
"""Two-layer GAT (KeypointGraph) on 8 Trainium2 NeuronCores.

Strategy (x-space aggregation, dst-sharded, window-pipelined):
 - GAT algebra: out[d] = (1/4)·Σ_h (Σ_e α_eh x[src_e]) @ W_h + b — the linear
   transform commutes with the α-weighted aggregation, so each core aggregates
   256-wide x rows (not 1024-wide h rows) and applies W once per 128-dst
   window. No replicated X@W, no H table round trip; gather traffic drops 4x.
 - Host: add self-loops, balance dst nodes into 8 cores x 9 windows of 128
   dst slots (window 8 half-filled); per (core,window) edges padded to 128-edge
   tiles; one-hot med/mde per tile (bf16); gather table TAB[n,260] = [x | x@Wa_src]
   (bf16); per-window dst attention ADW = x@Wa_dst permuted to slots.
 - Device (one NEFF per layer run, SPMD on 8 cores), per window:
   per-tile indirect row gather (Pool) -> xg; psa matmul (mde^T @ adw) gives
   per-edge a_dst; logits = gathered a_src + psa -> Prelu -> Exp (Act);
   den[d,h] via med^T @ exw (PE, ~free); rec = 1/den; per-edge rec via
   mde^T @ rec; exwn = exw·rec (normalized weights); mx = med x exwn broadcast
   (DVE/Act split); two accumulating zT matmuls per tile:
   zT[c,(h,d)] += xg[:,chalf]^T @ mx  (PSUM, 2 banks); after the window:
   zT -> SBUF bf16, 8 accumulating matmuls vs W chunks -> out[d,256] (+bias).
 - Host between layers: x2 = relu(y1); rebuild TAB/ADW with layer-2 weights and
   rerun the same NEFF.
"""

import sys

sys.path.insert(0, "/opt/trn_rl_repo")

import numpy as np
import ml_dtypes

import concourse.bass as bass
import concourse.mybir as mybir
import concourse.tile as tile
from concourse.bass_utils import run_bass_kernel_spmd

BF16 = ml_dtypes.bfloat16

B, K, F = 512, 17, 256
N = B * K              # 8704
HEADS, C = 4, 256
NCORES = 8
NPC = N // NCORES      # 1088 dst nodes per core
NWIN = 9               # 8 full 128-dst windows + 1 half window
NPAD = 8832            # gather table rows (8704 real + pad)
PADROW = N             # gather index for padding edges
ROWW = 260             # table row: 256 x cols + 4 a_src cols (520B)
NEG_SLOPE = 0.2

_cache = {}


def _split_multiwaits(nc):
    """This image's walrus supports only ONE sync-wait command per
    instruction; hoist extra waits onto prepended same-engine NoOps."""
    for f in nc.m.functions:
        for blk in f.blocks:
            old = blk.instructions
            new = []
            changed = False
            for inst in old:
                si = inst.sync_info
                if si is not None and len(si.on_wait) > 1:
                    waits = list(si.on_wait)
                    for k, w in enumerate(waits[:-1]):
                        new.append(
                            mybir.InstNoOp(
                                name=f"{inst.name}_wsplit{k}",
                                engine=inst.engine,
                                sync_info=mybir.SyncInfo(on_wait=[w], on_update=[]),
                                bass_nofuse=True,
                            )
                        )
                    inst.sync_info = mybir.SyncInfo(
                        on_wait=[waits[-1]], on_update=list(si.on_update)
                    )
                    changed = True
                new.append(inst)
            if changed:
                blk.instructions = new


def _build_layer_nc(tw, mx_sched):
    """One GAT layer, SPMD over 8 cores. tw[w]: tiles per window."""
    nc = bass.Bass(num_devices=NCORES)
    dt = mybir.dt
    twmax = max(tw)

    TAB = nc.dram_tensor("tab", [NPAD, ROWW], dt.bfloat16, kind="ExternalInput")
    WG = nc.dram_tensor("wg", [2, 128, HEADS * C], dt.bfloat16, kind="ExternalInput")
    BIAS = nc.dram_tensor("bias", [128, C], dt.float32, kind="ExternalInput")
    SRC = nc.dram_tensor("src", [128, NWIN * twmax], dt.int32, kind="ExternalInput")
    ADW = nc.dram_tensor("adw", [128, NWIN * 4], dt.bfloat16, kind="ExternalInput")
    MEDE = nc.dram_tensor(
        "mede", [NWIN, 128, twmax * 256], dt.bfloat16, kind="ExternalInput"
    )
    Y = nc.dram_tensor("y", [NWIN, 128, C], dt.float32, kind="ExternalOutput")

    with tile.TileContext(nc) as tc:
        with (
            tc.tile_pool(name="per", bufs=1) as per,
            tc.tile_pool(name="mw", bufs=4) as mw,
            tc.tile_pool(name="xg", bufs=4) as xgp,
            tc.tile_pool(name="sm", bufs=4) as sm,
            tc.tile_pool(name="mx", bufs=16) as mxp,
            tc.tile_pool(name="zs", bufs=3) as zs,
            tc.tile_pool(name="yt", bufs=3) as yt,
            tc.tile_pool(name="ppz", bufs=2, space="PSUM") as ppz,
            tc.tile_pool(name="pst", bufs=2, space="PSUM") as pst,
            tc.tile_pool(name="pot", bufs=2, space="PSUM") as pot,
        ):
            # ---- resident inputs ----
            wgs = []
            for k in range(2):
                w_ = per.tile([128, HEADS * C], dt.bfloat16, tag=f"wg{k}",
                              name=f"wg{k}")
                nc.scalar.dma_start(w_[:], WG[k])
                wgs.append(w_)
            bia = per.tile([128, C], dt.float32, tag="bias")
            nc.scalar.dma_start(bia[:], BIAS[:])
            srcm = per.tile([128, NWIN * twmax], dt.int32, tag="srcm", name="srcm")
            nc.sync.dma_start(srcm[:], SRC[:, :])
            adwsb = per.tile([128, NWIN * 4], dt.bfloat16, tag="adw", name="adwsb")
            nc.sync.dma_start(adwsb[:], ADW[:, :])

            for w in range(NWIN):
                twn = tw[w]
                medw = mw.tile([128, twmax * 256], dt.bfloat16, tag="medw",
                               name=f"medw{w}")
                nc.sync.dma_start(medw[:, 0 : twn * 256], MEDE[w, :, 0 : twn * 256])

                # ---- per-tile indirect row gathers (Pool) ----
                xgw = xgp.tile([128, twmax * ROWW], dt.bfloat16, tag="xgw",
                               name=f"xgw{w}")
                sidxw = srcm[:, w * twmax : w * twmax + twn]
                for t in range(twn):
                    nc.gpsimd.indirect_dma_start(
                        out=xgw[:, t * ROWW : (t + 1) * ROWW],
                        out_offset=None,
                        in_=TAB[:, :],
                        in_offset=bass.IndirectOffsetOnAxis(
                            ap=sidxw[:, t : t + 1], axis=0
                        ),
                    )

                # ---- strips: psa (per-edge a_dst), den, recp share one bank ----
                psT = pst.tile([128, 512], dt.float32, tag="pst", name=f"pst{w}")
                for t in range(twn):
                    nc.tensor.matmul(
                        psT[:, 4 * t : 4 * t + 4],
                        lhsT=medw[:, 256 * t + 128 : 256 * t + 256],
                        rhs=adwsb[:, 4 * w : 4 * w + 4],
                        start=True,
                        stop=True,
                    )

                # logits: eff = a_src(gathered) + psa; Prelu; Exp
                eff = sm.tile([128, 4 * twmax], dt.float32, tag="eff",
                              name=f"eff{w}")
                gv = xgw[:, 0 : twn * ROWW].rearrange(
                    "p (t c) -> p t c", t=twn, c=ROWW
                )[:, :, 256:260]
                nc.vector.tensor_add(
                    eff[:, 0 : 4 * twn].rearrange("p (t c) -> p t c", t=twn, c=4),
                    gv,
                    psT[:, 0 : 4 * twn].rearrange("p (t c) -> p t c", t=twn, c=4),
                )
                efl = sm.tile([128, 4 * twmax], dt.float32, tag="efl",
                              name=f"efl{w}")
                nc.scalar.activation(
                    efl[:, 0 : 4 * twn], eff[:, 0 : 4 * twn],
                    mybir.ActivationFunctionType.Prelu, alpha=NEG_SLOPE,
                )
                exwf = sm.tile([128, 4 * twmax], dt.float32, tag="exwf",
                               name=f"exwf{w}")
                nc.scalar.activation(
                    exwf[:, 0 : 4 * twn], efl[:, 0 : 4 * twn],
                    mybir.ActivationFunctionType.Exp,
                )
                exwb = sm.tile([128, 4 * twmax], dt.bfloat16, tag="exwb",
                               name=f"exwb{w}")
                nc.vector.tensor_copy(exwb[:, 0 : 4 * twn], exwf[:, 0 : 4 * twn])

                # den[d,h] = sum_e exw; rec = 1/den (bf16); recp[e,h] = rec[dst_e]
                for t in range(twn):
                    nc.tensor.matmul(
                        psT[:, 96:100],
                        lhsT=medw[:, 256 * t : 256 * t + 128],
                        rhs=exwb[:, 4 * t : 4 * t + 4],
                        start=(t == 0),
                        stop=(t == twn - 1),
                    )
                recb = sm.tile([128, 4], dt.bfloat16, tag="recb", name=f"recb{w}")
                with nc.allow_low_precision(reason="1/den to bf16 matmul rhs"):
                    nc.vector.reciprocal(recb[:], psT[:, 96:100])
                for t in range(twn):
                    nc.tensor.matmul(
                        psT[:, 128 + 4 * t : 128 + 4 * t + 4],
                        lhsT=medw[:, 256 * t + 128 : 256 * t + 256],
                        rhs=recb[:],
                        start=True,
                        stop=True,
                    )
                exwnf = sm.tile([128, 4 * twmax], dt.float32, tag="exwnf",
                                name=f"exwnf{w}")
                nc.vector.tensor_mul(
                    exwnf[:, 0 : 4 * twn], exwf[:, 0 : 4 * twn],
                    psT[:, 128 : 128 + 4 * twn],
                )

                # ---- weighted aggregation in x-space ----
                ztA = ppz.tile([128, 512], dt.float32, tag="ztA", name=f"ztA{w}")
                ztB = ppz.tile([128, 512], dt.float32, tag="ztB", name=f"ztB{w}")
                for t in range(twn):
                    first = t == 0
                    last = t == twn - 1
                    mx = mxp.tile([128, 512], dt.bfloat16, tag="mx",
                                  name=f"mx_{w}_{t}")
                    if mx_sched[w][t] == 0:
                        # DVE: one fused 4-head broadcast multiply
                        nc.vector.tensor_mul(
                            mx[:].rearrange("p (h c) -> p h c", h=4, c=128),
                            medw[:, 256 * t : 256 * t + 128].unsqueeze(1)
                                .to_broadcast([128, 4, 128]),
                            exwnf[:, 4 * t : 4 * t + 4].unsqueeze(2)
                                .to_broadcast([128, 4, 128]),
                        )
                    else:
                        # Act: per-head scalar-scale copies
                        for h in range(HEADS):
                            nc.scalar.mul(
                                mx[:, 128 * h : 128 * (h + 1)],
                                medw[:, 256 * t : 256 * t + 128],
                                exwnf[:, 4 * t + h : 4 * t + h + 1],
                            )
                    nc.tensor.matmul(
                        ztA[:],
                        lhsT=xgw[:, t * ROWW : t * ROWW + 128],
                        rhs=mx[:],
                        start=first,
                        stop=last,
                    )
                    nc.tensor.matmul(
                        ztB[:],
                        lhsT=xgw[:, t * ROWW + 128 : t * ROWW + 256],
                        rhs=mx[:],
                        start=first,
                        stop=last,
                    )

                # ---- per-window transform: out = sum_h z_h @ W_h/4 + b ----
                zsa = zs.tile([128, 512], dt.bfloat16, tag="zsa", name=f"zsa{w}")
                nc.vector.tensor_copy(zsa[:], ztA[:])
                zsb = zs.tile([128, 512], dt.bfloat16, tag="zsb", name=f"zsb{w}")
                nc.scalar.copy(zsb[:], ztB[:])
                outw = pot.tile([128, C], dt.float32, tag="outw", name=f"outw{w}")
                for h in range(HEADS):
                    nc.tensor.matmul(
                        outw[:],
                        lhsT=zsa[:, 128 * h : 128 * (h + 1)],
                        rhs=wgs[0][:, C * h : C * (h + 1)],
                        start=(h == 0),
                        stop=False,
                    )
                    nc.tensor.matmul(
                        outw[:],
                        lhsT=zsb[:, 128 * h : 128 * (h + 1)],
                        rhs=wgs[1][:, C * h : C * (h + 1)],
                        start=False,
                        stop=(h == HEADS - 1),
                    )
                yacc = yt.tile([128, C], dt.float32, tag="yacc", name=f"yacc{w}")
                nc.vector.tensor_add(yacc[:], outw[:], bia[:])
                nc.sync.dma_start(Y[w], yacc[:])

    _split_multiwaits(nc)
    return nc


def _host_prep(edge_index):
    ei = np.asarray(edge_index).astype(np.int64)
    loop = np.arange(N, dtype=np.int64)
    src = np.concatenate([ei[0], loop])
    dst = np.concatenate([ei[1], loop])

    # ---- balance dsts into (core, window) buckets by total degree ----
    NBK = NCORES * NWIN
    deg = np.bincount(dst, minlength=N).astype(np.int64)
    cap = np.where(np.arange(NBK) % NWIN == NWIN - 1, 64, 128)
    targ = deg.sum() / (N / 128.0)
    targ_b = targ * (cap / 128.0)
    order = np.argsort(-deg, kind="stable")
    L = np.zeros(NBK, np.float64)
    nfill = np.zeros(NBK, np.int64)
    bid = np.zeros(N, np.int64)
    for d in order:
        score = (L + deg[d]) - targ_b
        score[nfill >= cap] = np.inf
        b = int(np.argmin(score))
        L[b] += deg[d]
        bid[d] = b
        nfill[b] += 1

    # swap refinement: drive every bucket load to <= ceil-target so tiles/window
    # hit the minimum (full buckets 17*128 edges, half bucket 9*128-64)
    members = [list(np.where(bid == b)[0]) for b in range(NBK)]
    limit = np.where(np.arange(NBK) % NWIN == NWIN - 1, 9 * 128 - 64, 17 * 128)
    limit = limit.astype(np.float64)
    for _ in range(5000):
        over = L - limit
        b1 = int(np.argmax(over))
        need = over[b1]
        if need <= 0:
            break
        m1 = members[b1]
        deg1 = deg[np.array(m1)]
        best = None
        for b2 in np.argsort(over)[:8]:
            b2 = int(b2)
            if b2 == b1 or over[b2] >= 0:
                continue
            m2 = members[b2]
            deg2 = deg[np.array(m2)]
            room = -over[b2]
            # delta in [1, min(need ceil-slack?, room)]; aim delta ~= need
            dmat = deg1[:, None] - deg2[None, :]
            ok = (dmat > 0) & (dmat <= room)
            if not ok.any():
                continue
            dm = np.where(ok, np.abs(dmat - need), np.inf)
            i, k = np.unravel_index(int(dm.argmin()), dm.shape)
            cand = (float(dm[i, k]), int(dmat[i, k]), b2, int(m1[i]), int(m2[k]))
            if best is None or cand[0] < best[0]:
                best = cand
        if best is None:
            break
        _, delta, b2, d1, d2 = best
        members[b1].remove(d1)
        members[b2].remove(d2)
        members[b1].append(d2)
        members[b2].append(d1)
        bid[d1], bid[d2] = b2, b1
        L[b1] -= delta
        L[b2] += delta

    pj = np.zeros(N, np.int64)
    pw = np.zeros(N, np.int64)
    pslot = np.zeros(N, np.int64)
    for b in range(NBK):
        for s, d in enumerate(members[b]):
            pj[d] = b // NWIN
            pw[d] = b % NWIN
            pslot[d] = s

    ecore = pj[dst]
    ewin = pw[dst]
    dstw = pslot[dst]

    cnt = np.zeros((NCORES, NWIN), np.int64)
    for j in range(NCORES):
        m = ecore == j
        for w in range(NWIN):
            cnt[j, w] = int((m & (ewin == w)).sum())
    # pad dst slots (window 8 has 64 empty slots) get pad self-edges so that
    # den >= 1 everywhere; reserve room for them when sizing tiles
    npad_dst = np.zeros(NWIN, np.int64)
    npad_dst[NWIN - 1] = 128 - 64
    tw = [int(np.ceil((cnt[:, w].max() + npad_dst[w]) / 128)) for w in range(NWIN)]
    twmax = max(tw)
    T = sum(tw)

    srcw = np.full((NCORES, NWIN, 128, twmax), PADROW, np.int32)
    dstwin = np.full((NCORES, NWIN, 128, twmax), -1, np.int64)
    for j in range(NCORES):
        m = ecore == j
        for w in range(NWIN):
            mw_ = m & (ewin == w)
            s = src[mw_]
            d = dstw[mw_]
            cnte = len(s)
            es = np.arange(cnte)
            srcw[j, w, es % 128, es // 128] = s.astype(np.int32)
            dstwin[j, w, es % 128, es // 128] = d
            # pad self-edges for empty dst slots
            base = int(NWIN == 0)
            nreal = int(nfill[j * NWIN + w])
            pads = np.arange(nreal, 128 if w < NWIN - 1 else 128)
            pads = pads[pads < 128]
            if w == NWIN - 1:
                pads = np.arange(64, 128)
            else:
                pads = np.arange(nreal, 128)
            ep = np.arange(cnte, cnte + len(pads))
            assert ep.max(initial=-1) < twmax * 128
            srcw[j, w, ep % 128, ep // 128] = PADROW
            dstwin[j, w, ep % 128, ep // 128] = pads

    iota = np.arange(128)
    med = (dstwin[..., None] == iota[None, None, None, None, :]).astype(BF16)
    mde = med.transpose(0, 1, 4, 3, 2).copy()
    mede = np.empty((NCORES, NWIN, 128, twmax, 256), BF16)
    mede[..., 0:128] = med
    mede[..., 128:256] = mde
    mede = mede.reshape(NCORES, NWIN, 128, twmax * 256).copy()

    srcw = srcw.transpose(0, 2, 1, 3).reshape(NCORES, 128, NWIN * twmax).copy()
    return tw, T, srcw, mede, (pj, pw, pslot)


def _fold_attn(W, a):
    """Wa = W @ a per head: [F_in, HEADS]."""
    W64 = np.asarray(W, np.float64)
    A = np.asarray(a, np.float64)
    Wh = W64.reshape(W64.shape[0], HEADS, C)
    return (Wh * A[None]).sum(-1)  # [F_in, HEADS]


def _layer_inputs(x, Wl, a_src, a_dst, bias, placement):
    """Host-side per-layer tensors: TAB, ADW, WG, BIAS."""
    pj, pw, pslot = placement
    x64 = np.asarray(x, np.float64)
    asrc = x64 @ _fold_attn(Wl, a_src)          # [N, 4]
    adst = x64 @ _fold_attn(Wl, a_dst)          # [N, 4]
    tab = np.zeros((NPAD, ROWW), np.float32)
    tab[:N, 0:256] = np.asarray(x, np.float32)
    tab[:N, 256:260] = asrc
    tabb = tab.astype(BF16)
    adw = np.zeros((NCORES, NWIN, 128, 4), np.float32)
    adw[pj, pw, pslot] = adst
    adw = adw.transpose(0, 2, 1, 3).reshape(NCORES, 128, NWIN * 4).astype(BF16)
    wg = (np.asarray(Wl, np.float64) * 0.25).astype(BF16).reshape(2, 128, HEADS * C)
    bias_b = np.broadcast_to(
        np.asarray(bias, np.float32)[None, :], (128, C)
    ).copy()
    return tabb, adw, wg, bias_b


def _run_layer(nc, tabb, adw, wg, bias_b, srcw, mede, placement):
    in_maps = []
    for j in range(NCORES):
        in_maps.append(
            {
                "tab": tabb,
                "wg": wg,
                "bias": bias_b,
                "src": srcw[j],
                "adw": adw[j],
                "mede": mede[j],
            }
        )
    res = run_bass_kernel_spmd(nc, in_maps, core_ids=list(range(NCORES)))
    pj, pw, pslot = placement
    yall = np.stack([res.results[j]["y"] for j in range(NCORES)])  # [NC,NWIN,128,C]
    y = yall[pj, pw, pslot].astype(np.float32)
    return y


def _mx_schedule(tw):
    """Per (window, tile) engine for the mx broadcast: 0=DVE, 1=Act."""
    return [[0 if (t % 3) != 2 else 1 for t in range(tw[w])] for w in range(NWIN)]


def kernel(kpt_feature, edge_index, W1, a_src1, a_dst1, b1, W2, a_src2, a_dst2, b2):
    key = "k"
    if key not in _cache:
        tw, T, srcw, mede, placement = _host_prep(edge_index)
        nc = _build_layer_nc(tw, _mx_schedule(tw))
        _cache[key] = (nc, tw, T, srcw, mede, placement)
    nc, tw, T, srcw, mede, placement = _cache[key]

    x1 = np.asarray(kpt_feature, np.float32).reshape(N, F)
    y1 = _run_layer(
        nc, *_layer_inputs(x1, W1, a_src1, a_dst1, b1, placement), srcw, mede,
        placement,
    )
    x2 = np.maximum(y1, 0.0)
    y2 = _run_layer(
        nc, *_layer_inputs(x2, W2, a_src2, a_dst2, b2, placement), srcw, mede,
        placement,
    )
    return y2.reshape(B, K, F).astype(np.float32)


# revision 15
# speedup vs baseline: 1.6327x; 1.0429x over previous
"""Two-layer GAT (KeypointGraph) on 8 Trainium2 NeuronCores.

Strategy (x-space aggregation, dst-sharded, window-pipelined):
 - GAT algebra: out[d] = (1/4)·Σ_h (Σ_e α_eh x[src_e]) @ W_h + b — the linear
   transform commutes with the α-weighted aggregation, so each core aggregates
   256-wide x rows (not 1024-wide h rows) and applies W once per 128-dst
   window. No replicated X@W, no H table round trip; gather traffic drops 4x.
 - Host: add self-loops, balance dst nodes into 8 cores x 9 windows of 128
   dst slots (window 8 half-filled); per (core,window) edges padded to 128-edge
   tiles; one-hot med/mde per tile (bf16); gather table TAB[n,260] = [x | x@Wa_src]
   (bf16); per-window dst attention ADW = x@Wa_dst permuted to slots.
 - Device (one NEFF per layer run, SPMD on 8 cores), per window:
   per-tile indirect row gather (Pool) -> xg; psa matmul (mde^T @ adw) gives
   per-edge a_dst; logits = gathered a_src + psa -> Prelu -> Exp (Act);
   den[d,h] via med^T @ exw (PE, ~free); rec = 1/den; per-edge rec via
   mde^T @ rec; exwn = exw·rec (normalized weights); mx = med x exwn broadcast
   (DVE/Act split); two accumulating zT matmuls per tile:
   zT[c,(h,d)] += xg[:,chalf]^T @ mx  (PSUM, 2 banks); after the window:
   zT -> SBUF bf16, 8 accumulating matmuls vs W chunks -> out[d,256] (+bias).
 - Host between layers: x2 = relu(y1); rebuild TAB/ADW with layer-2 weights and
   rerun the same NEFF.
"""

import sys

sys.path.insert(0, "/opt/trn_rl_repo")

import numpy as np
import ml_dtypes

import concourse.bass as bass
import concourse.mybir as mybir
import concourse.tile as tile
from concourse.bass_utils import run_bass_kernel_spmd

BF16 = ml_dtypes.bfloat16

B, K, F = 512, 17, 256
N = B * K              # 8704
HEADS, C = 4, 256
NCORES = 8
NPC = N // NCORES      # 1088 dst nodes per core
NWIN = 10              # window dst caps: small first (fast pipeline fill),
CAPS = [64] + [128] * 7 + [64, 64]   # two small last (short drain tail)
NPAD = 8832            # gather table rows (8704 real + pad)
PADROW = N             # gather index for padding edges
ROWW = 260             # table row: 256 x cols + 4 a_src cols (520B)
NEG_SLOPE = 0.2

_cache = {}


def _split_multiwaits(nc):
    """This image's walrus supports only ONE sync-wait command per
    instruction; hoist extra waits onto prepended same-engine NoOps."""
    for f in nc.m.functions:
        for blk in f.blocks:
            old = blk.instructions
            new = []
            changed = False
            for inst in old:
                si = inst.sync_info
                if si is not None and len(si.on_wait) > 1:
                    waits = list(si.on_wait)
                    for k, w in enumerate(waits[:-1]):
                        new.append(
                            mybir.InstNoOp(
                                name=f"{inst.name}_wsplit{k}",
                                engine=inst.engine,
                                sync_info=mybir.SyncInfo(on_wait=[w], on_update=[]),
                                bass_nofuse=True,
                            )
                        )
                    inst.sync_info = mybir.SyncInfo(
                        on_wait=[waits[-1]], on_update=list(si.on_update)
                    )
                    changed = True
                new.append(inst)
            if changed:
                blk.instructions = new


def _build_layer_nc(tw, mx_sched):
    """One GAT layer, SPMD over 8 cores. tw[w]: tiles per window."""
    nc = bass.Bass(num_devices=NCORES)
    dt = mybir.dt
    twmax = max(tw)

    TAB = nc.dram_tensor("tab", [NPAD, ROWW], dt.bfloat16, kind="ExternalInput")
    WG = nc.dram_tensor("wg", [2, 128, HEADS * C], dt.bfloat16, kind="ExternalInput")
    BIAS = nc.dram_tensor("bias", [1, C], dt.bfloat16, kind="ExternalInput")
    XD = nc.dram_tensor("xd", [NWIN, 128, ROWW], dt.bfloat16, kind="ExternalInput")
    SRC = nc.dram_tensor("src", [128, NWIN * twmax], dt.int32, kind="ExternalInput")
    ADW = nc.dram_tensor("adw", [128, NWIN * 4], dt.bfloat16, kind="ExternalInput")
    MEDE = nc.dram_tensor(
        "mede", [NWIN, 128, twmax * 256], dt.bfloat16, kind="ExternalInput"
    )
    Y = nc.dram_tensor("y", [NWIN, 128, C], dt.float32, kind="ExternalOutput")

    with tile.TileContext(nc) as tc:
        with (
            tc.tile_pool(name="per", bufs=1) as per,
            tc.tile_pool(name="mw", bufs=4) as mw,
            tc.tile_pool(name="xg", bufs=4) as xgp,
            tc.tile_pool(name="sm", bufs=4) as sm,
            tc.tile_pool(name="mx", bufs=16) as mxp,
            tc.tile_pool(name="zs", bufs=3) as zs,
            tc.tile_pool(name="yt", bufs=3) as yt,
            tc.tile_pool(name="ppz", bufs=2, space="PSUM") as ppz,
            tc.tile_pool(name="pst", bufs=2, space="PSUM") as pst,
            tc.tile_pool(name="pot", bufs=2, space="PSUM") as pot,
        ):
            # ---- resident inputs ----
            wgs = []
            for k in range(2):
                w_ = per.tile([128, HEADS * C], dt.bfloat16, tag=f"wg{k}",
                              name=f"wg{k}")
                nc.scalar.dma_start(w_[:], WG[k])
                wgs.append(w_)
            bia = per.tile([1, C], dt.bfloat16, tag="bias")
            nc.scalar.dma_start(bia[:], BIAS[:])
            ones = per.tile([1, 128], dt.bfloat16, tag="ones")
            nc.vector.memset(ones[:], 1.0)
            srcm = per.tile([128, NWIN * twmax], dt.int32, tag="srcm", name="srcm")
            nc.sync.dma_start(srcm[:], SRC[:, :])
            adwsb = per.tile([128, NWIN * 4], dt.bfloat16, tag="adw", name="adwsb")
            nc.sync.dma_start(adwsb[:], ADW[:, :])

            for w in range(NWIN):
                twn = tw[w]
                medw = mw.tile([128, twmax * 256], dt.bfloat16, tag="medw",
                               name=f"medw{w}")
                nc.sync.dma_start(medw[:, 0 : twn * 256], MEDE[w, :, 0 : twn * 256])

                # ---- per-tile indirect row gathers (Pool) ----
                xgw = xgp.tile([128, twmax * ROWW], dt.bfloat16, tag="xgw",
                               name=f"xgw{w}")
                sidxw = srcm[:, w * twmax : w * twmax + twn]
                for t in range(twn - 1):
                    nc.gpsimd.indirect_dma_start(
                        out=xgw[:, t * ROWW : (t + 1) * ROWW],
                        out_offset=None,
                        in_=TAB[:, :],
                        in_offset=bass.IndirectOffsetOnAxis(
                            ap=sidxw[:, t : t + 1], axis=0
                        ),
                    )
                # self-loop tile: window dst rows direct from host table
                nc.scalar.dma_start(
                    xgw[:, (twn - 1) * ROWW : twn * ROWW], XD[w]
                )

                # ---- strips: psa (per-edge a_dst), den, recp share one bank ----
                psT = pst.tile([128, 512], dt.float32, tag="pst", name=f"pst{w}")
                for t in range(twn):
                    nc.tensor.matmul(
                        psT[:, 4 * t : 4 * t + 4],
                        lhsT=medw[:, 256 * t + 128 : 256 * t + 256],
                        rhs=adwsb[:, 4 * w : 4 * w + 4],
                        start=True,
                        stop=True,
                    )

                # logits: eff = a_src(gathered) + psa; Prelu; Exp
                eff = sm.tile([128, 4 * twmax], dt.float32, tag="eff",
                              name=f"eff{w}")
                gv = xgw[:, 0 : twn * ROWW].rearrange(
                    "p (t c) -> p t c", t=twn, c=ROWW
                )[:, :, 256:260]
                nc.vector.tensor_add(
                    eff[:, 0 : 4 * twn].rearrange("p (t c) -> p t c", t=twn, c=4),
                    gv,
                    psT[:, 0 : 4 * twn].rearrange("p (t c) -> p t c", t=twn, c=4),
                )
                efl = sm.tile([128, 4 * twmax], dt.float32, tag="efl",
                              name=f"efl{w}")
                nc.scalar.activation(
                    efl[:, 0 : 4 * twn], eff[:, 0 : 4 * twn],
                    mybir.ActivationFunctionType.Prelu, alpha=NEG_SLOPE,
                )
                exwf = sm.tile([128, 4 * twmax], dt.float32, tag="exwf",
                               name=f"exwf{w}")
                nc.scalar.activation(
                    exwf[:, 0 : 4 * twn], efl[:, 0 : 4 * twn],
                    mybir.ActivationFunctionType.Exp,
                )
                exwb = sm.tile([128, 4 * twmax], dt.bfloat16, tag="exwb",
                               name=f"exwb{w}")
                nc.vector.tensor_copy(exwb[:, 0 : 4 * twn], exwf[:, 0 : 4 * twn])

                # den[d,h] = sum_e exw; rec = 1/den (bf16); recp[e,h] = rec[dst_e]
                for t in range(twn):
                    nc.tensor.matmul(
                        psT[:, 96:100],
                        lhsT=medw[:, 256 * t : 256 * t + 128],
                        rhs=exwb[:, 4 * t : 4 * t + 4],
                        start=(t == 0),
                        stop=(t == twn - 1),
                    )
                recb = sm.tile([128, 4], dt.bfloat16, tag="recb", name=f"recb{w}")
                with nc.allow_low_precision(reason="1/den to bf16 matmul rhs"):
                    nc.vector.reciprocal(recb[:], psT[:, 96:100])
                for t in range(twn):
                    nc.tensor.matmul(
                        psT[:, 128 + 4 * t : 128 + 4 * t + 4],
                        lhsT=medw[:, 256 * t + 128 : 256 * t + 256],
                        rhs=recb[:],
                        start=True,
                        stop=True,
                    )
                exwnf = sm.tile([128, 4 * twmax], dt.float32, tag="exwnf",
                                name=f"exwnf{w}")
                nc.vector.tensor_mul(
                    exwnf[:, 0 : 4 * twn], exwf[:, 0 : 4 * twn],
                    psT[:, 128 : 128 + 4 * twn],
                )

                # ---- weighted aggregation in x-space ----
                ztA = ppz.tile([128, 512], dt.float32, tag="ztA", name=f"ztA{w}")
                ztB = ppz.tile([128, 512], dt.float32, tag="ztB", name=f"ztB{w}")
                for t in range(twn):
                    first = t == 0
                    last = t == twn - 1
                    mx = mxp.tile([128, 512], dt.bfloat16, tag="mx",
                                  name=f"mx_{w}_{t}")
                    if mx_sched[w][t] == 0:
                        # DVE: per-head scalar-ptr multiplies (2x fast path)
                        for h in range(HEADS):
                            nc.vector.tensor_scalar_mul(
                                mx[:, 128 * h : 128 * (h + 1)],
                                medw[:, 256 * t : 256 * t + 128],
                                exwnf[:, 4 * t + h : 4 * t + h + 1],
                            )
                    else:
                        # Act: per-head scalar-scale copies
                        for h in range(HEADS):
                            nc.scalar.mul(
                                mx[:, 128 * h : 128 * (h + 1)],
                                medw[:, 256 * t : 256 * t + 128],
                                exwnf[:, 4 * t + h : 4 * t + h + 1],
                            )
                    nc.tensor.matmul(
                        ztA[:],
                        lhsT=xgw[:, t * ROWW : t * ROWW + 128],
                        rhs=mx[:],
                        start=first,
                        stop=last,
                    )
                    nc.tensor.matmul(
                        ztB[:],
                        lhsT=xgw[:, t * ROWW + 128 : t * ROWW + 256],
                        rhs=mx[:],
                        start=first,
                        stop=last,
                    )

                # ---- per-window transform: out = sum_h z_h @ W_h/4 + b ----
                zsa = zs.tile([128, 512], dt.bfloat16, tag="zsa", name=f"zsa{w}")
                nc.vector.tensor_copy(zsa[:], ztA[:])
                zsb = zs.tile([128, 512], dt.bfloat16, tag="zsb", name=f"zsb{w}")
                nc.scalar.copy(zsb[:], ztB[:])
                outw = pot.tile([128, C], dt.float32, tag="outw", name=f"outw{w}")
                nc.tensor.matmul(
                    outw[:], lhsT=ones[:], rhs=bia[:], start=True, stop=False,
                )
                for h in range(HEADS):
                    nc.tensor.matmul(
                        outw[:],
                        lhsT=zsa[:, 128 * h : 128 * (h + 1)],
                        rhs=wgs[0][:, C * h : C * (h + 1)],
                        start=False,
                        stop=False,
                    )
                    nc.tensor.matmul(
                        outw[:],
                        lhsT=zsb[:, 128 * h : 128 * (h + 1)],
                        rhs=wgs[1][:, C * h : C * (h + 1)],
                        start=False,
                        stop=(h == HEADS - 1),
                    )
                yacc = yt.tile([128, C], dt.float32, tag="yacc", name=f"yacc{w}")
                if w % 2 == 0:
                    nc.vector.tensor_copy(yacc[:], outw[:])
                else:
                    nc.scalar.copy(yacc[:], outw[:])
                nc.sync.dma_start(Y[w], yacc[:])

    _split_multiwaits(nc)
    return nc


def _host_prep(edge_index):
    ei = np.asarray(edge_index).astype(np.int64)
    loop = np.arange(N, dtype=np.int64)
    src = np.concatenate([ei[0], loop])
    dst = np.concatenate([ei[1], loop])

    # ---- balance dsts into (core, window) buckets by total degree ----
    NBK = NCORES * NWIN
    deg = np.bincount(dst, minlength=N).astype(np.int64)
    cap = np.array([CAPS[b % NWIN] for b in range(NBK)], np.int64)
    targ = deg.sum() / (N / 128.0)
    targ_b = targ * (cap / 128.0)
    order = np.argsort(-deg, kind="stable")
    L = np.zeros(NBK, np.float64)
    nfill = np.zeros(NBK, np.int64)
    bid = np.zeros(N, np.int64)
    for d in order:
        score = (L + deg[d]) - targ_b
        score[nfill >= cap] = np.inf
        b = int(np.argmin(score))
        L[b] += deg[d]
        bid[d] = b
        nfill[b] += 1

    # swap refinement: drive every bucket load to <= ceil-target so tiles/window
    # hit the minimum (full buckets 17*128 edges, half bucket 9*128-64)
    members = [list(np.where(bid == b)[0]) for b in range(NBK)]
    # total-degree limits so gathered (non-self) tiles hit the minimum count
    limit = np.array(
        [16 * 128 + CAPS[b % NWIN] if CAPS[b % NWIN] == 128
         else 8 * 128 + CAPS[b % NWIN] for b in range(NBK)], np.float64)
    for _ in range(5000):
        over = L - limit
        b1 = int(np.argmax(over))
        need = over[b1]
        if need <= 0:
            break
        m1 = members[b1]
        deg1 = deg[np.array(m1)]
        best = None
        for b2 in np.argsort(over)[:8]:
            b2 = int(b2)
            if b2 == b1 or over[b2] >= 0:
                continue
            m2 = members[b2]
            deg2 = deg[np.array(m2)]
            room = -over[b2]
            # delta in [1, min(need ceil-slack?, room)]; aim delta ~= need
            dmat = deg1[:, None] - deg2[None, :]
            ok = (dmat > 0) & (dmat <= room)
            if not ok.any():
                continue
            dm = np.where(ok, np.abs(dmat - need), np.inf)
            i, k = np.unravel_index(int(dm.argmin()), dm.shape)
            cand = (float(dm[i, k]), int(dmat[i, k]), b2, int(m1[i]), int(m2[k]))
            if best is None or cand[0] < best[0]:
                best = cand
        if best is None:
            break
        _, delta, b2, d1, d2 = best
        members[b1].remove(d1)
        members[b2].remove(d2)
        members[b1].append(d2)
        members[b2].append(d1)
        bid[d1], bid[d2] = b2, b1
        L[b1] -= delta
        L[b2] += delta

    pj = np.zeros(N, np.int64)
    pw = np.zeros(N, np.int64)
    pslot = np.zeros(N, np.int64)
    for b in range(NBK):
        for s, d in enumerate(members[b]):
            pj[d] = b // NWIN
            pw[d] = b % NWIN
            pslot[d] = s

    # only the E original edges go through the gather path; the N explicit
    # self-loops become one identity tile per window fed by a direct DMA
    nsrc = src[: len(ei[0])]
    ndst = dst[: len(ei[0])]
    ecore_n = pj[ndst]
    ewin_n = pw[ndst]
    dstw_n = pslot[ndst]
    cnt = np.zeros((NCORES, NWIN), np.int64)
    for j in range(NCORES):
        m = ecore_n == j
        for w in range(NWIN):
            cnt[j, w] = int((m & (ewin_n == w)).sum())
    tw = [int(np.ceil(cnt[:, w].max() / 128)) + 1 for w in range(NWIN)]
    twmax = max(tw)
    T = sum(tw)

    srcw = np.full((NCORES, NWIN, 128, twmax), PADROW, np.int32)
    dstwin = np.full((NCORES, NWIN, 128, twmax), -1, np.int64)
    adix = np.full((NCORES, NWIN, 128), PADROW, np.int32)
    dstid = np.arange(N, dtype=np.int64)
    adix[pj, pw, pslot] = dstid
    for j in range(NCORES):
        m = ecore_n == j
        for w in range(NWIN):
            mw_ = m & (ewin_n == w)
            s = nsrc[mw_]
            d = dstw_n[mw_]
            cnte = len(s)
            es = np.arange(cnte)
            srcw[j, w, es % 128, es // 128] = s.astype(np.int32)
            dstwin[j, w, es % 128, es // 128] = d
            # identity self tile (covers real self-loops AND pad dst slots)
            dstwin[j, w, :, tw[w] - 1] = np.arange(128)

    iota = np.arange(128)
    med = (dstwin[..., None] == iota[None, None, None, None, :]).astype(BF16)
    mde = med.transpose(0, 1, 4, 3, 2).copy()
    mede = np.empty((NCORES, NWIN, 128, twmax, 256), BF16)
    mede[..., 0:128] = med
    mede[..., 128:256] = mde
    mede = mede.reshape(NCORES, NWIN, 128, twmax * 256).copy()

    srcw = srcw.transpose(0, 2, 1, 3).reshape(NCORES, 128, NWIN * twmax).copy()
    return tw, T, srcw, mede, adix, (pj, pw, pslot)


def _fold_attn(W, a):
    """Wa = W @ a per head: [F_in, HEADS]."""
    W64 = np.asarray(W, np.float64)
    A = np.asarray(a, np.float64)
    Wh = W64.reshape(W64.shape[0], HEADS, C)
    return (Wh * A[None]).sum(-1)  # [F_in, HEADS]


def _layer_inputs(x, Wl, a_src, a_dst, bias, adix, placement):
    """Host-side per-layer tensors: TAB, ADW, WG, BIAS, XD."""
    pj, pw, pslot = placement
    x64 = np.asarray(x, np.float64)
    asrc = x64 @ _fold_attn(Wl, a_src)          # [N, 4]
    adst = x64 @ _fold_attn(Wl, a_dst)          # [N, 4]
    tab = np.zeros((NPAD, ROWW), np.float32)
    tab[:N, 0:256] = np.asarray(x, np.float32)
    tab[:N, 256:260] = asrc
    tabb = tab.astype(BF16)
    xd = tabb[adix]                              # [NC, NWIN, 128, ROWW]
    adw = np.zeros((NCORES, NWIN, 128, 4), np.float32)
    adw[pj, pw, pslot] = adst
    adw = adw.transpose(0, 2, 1, 3).reshape(NCORES, 128, NWIN * 4).astype(BF16)
    wg = (np.asarray(Wl, np.float64) * 0.25).astype(BF16).reshape(2, 128, HEADS * C)
    bias_b = np.asarray(bias, np.float32)[None, :].astype(BF16)
    return tabb, adw, wg, bias_b, xd


def _run_layer(nc, tabb, adw, wg, bias_b, xd, srcw, mede, placement):
    in_maps = []
    for j in range(NCORES):
        in_maps.append(
            {
                "tab": tabb,
                "wg": wg,
                "bias": bias_b,
                "xd": xd[j],
                "src": srcw[j],
                "adw": adw[j],
                "mede": mede[j],
            }
        )
    res = run_bass_kernel_spmd(nc, in_maps, core_ids=list(range(NCORES)))
    pj, pw, pslot = placement
    yall = np.stack([res.results[j]["y"] for j in range(NCORES)])  # [NC,NWIN,128,C]
    y = yall[pj, pw, pslot].astype(np.float32)
    return y


def _mx_schedule(tw):
    """Per (window, tile) engine for the mx broadcast: 0=DVE, 1=Act."""
    return [[0 if (t % 4) != 3 else 1 for t in range(tw[w])] for w in range(NWIN)]


def kernel(kpt_feature, edge_index, W1, a_src1, a_dst1, b1, W2, a_src2, a_dst2, b2):
    key = "k"
    if key not in _cache:
        tw, T, srcw, mede, adix, placement = _host_prep(edge_index)
        nc = _build_layer_nc(tw, _mx_schedule(tw))
        _cache[key] = (nc, tw, T, srcw, mede, adix, placement)
    nc, tw, T, srcw, mede, adix, placement = _cache[key]

    x1 = np.asarray(kpt_feature, np.float32).reshape(N, F)
    y1 = _run_layer(
        nc, *_layer_inputs(x1, W1, a_src1, a_dst1, b1, adix, placement),
        srcw, mede, placement,
    )
    x2 = np.maximum(y1, 0.0)
    y2 = _run_layer(
        nc, *_layer_inputs(x2, W2, a_src2, a_dst2, b2, adix, placement),
        srcw, mede, placement,
    )
    return y2.reshape(B, K, F).astype(np.float32)


# revision 17
# speedup vs baseline: 1.9196x; 1.1757x over previous
"""Two-layer GAT (KeypointGraph) on 8 Trainium2 NeuronCores.

Strategy (x-space aggregation, dst-sharded, window-pipelined):
 - GAT algebra: out[d] = (1/4)·Σ_h (Σ_e α_eh x[src_e]) @ W_h + b — the linear
   transform commutes with the α-weighted aggregation, so each core aggregates
   256-wide x rows (not 1024-wide h rows) and applies W once per 128-dst
   window. No replicated X@W, no H table round trip; gather traffic drops 4x.
 - Host: add self-loops, balance dst nodes into 8 cores x 9 windows of 128
   dst slots (window 8 half-filled); per (core,window) edges padded to 128-edge
   tiles; one-hot med/mde per tile (bf16); gather table TAB[n,260] = [x | x@Wa_src]
   (bf16); per-window dst attention ADW = x@Wa_dst permuted to slots.
 - Device (one NEFF per layer run, SPMD on 8 cores), per window:
   per-tile indirect row gather (Pool) -> xg; psa matmul (mde^T @ adw) gives
   per-edge a_dst; logits = gathered a_src + psa -> Prelu -> Exp (Act);
   den[d,h] via med^T @ exw (PE, ~free); rec = 1/den; per-edge rec via
   mde^T @ rec; exwn = exw·rec (normalized weights); mx = med x exwn broadcast
   (DVE/Act split); two accumulating zT matmuls per tile:
   zT[c,(h,d)] += xg[:,chalf]^T @ mx  (PSUM, 2 banks); after the window:
   zT -> SBUF bf16, 8 accumulating matmuls vs W chunks -> out[d,256] (+bias).
 - Host between layers: x2 = relu(y1); rebuild TAB/ADW with layer-2 weights and
   rerun the same NEFF.
"""

import sys

sys.path.insert(0, "/opt/trn_rl_repo")

import numpy as np
import ml_dtypes

import concourse.bass as bass
import concourse.mybir as mybir
import concourse.tile as tile
from concourse.bass_utils import run_bass_kernel_spmd

BF16 = ml_dtypes.bfloat16

B, K, F = 512, 17, 256
N = B * K              # 8704
HEADS, C = 4, 256
NCORES = 8
NPC = N // NCORES      # 1088 dst nodes per core
NWIN = 17              # uniform 64-dst windows: halved mx/zT column space
DW = 64                # dst slots per window
CAPS = [DW] * NWIN
NPAD = 8832            # gather table rows (8704 real + pad)
PADROW = N             # gather index for padding edges
ROWW = 260             # table row: 256 x cols + 4 a_src cols (520B)
NEG_SLOPE = 0.2

_cache = {}


def _split_multiwaits(nc):
    """This image's walrus supports only ONE sync-wait command per
    instruction; hoist extra waits onto prepended same-engine NoOps."""
    for f in nc.m.functions:
        for blk in f.blocks:
            old = blk.instructions
            new = []
            changed = False
            for inst in old:
                si = inst.sync_info
                if si is not None and len(si.on_wait) > 1:
                    waits = list(si.on_wait)
                    for k, w in enumerate(waits[:-1]):
                        new.append(
                            mybir.InstNoOp(
                                name=f"{inst.name}_wsplit{k}",
                                engine=inst.engine,
                                sync_info=mybir.SyncInfo(on_wait=[w], on_update=[]),
                                bass_nofuse=True,
                            )
                        )
                    inst.sync_info = mybir.SyncInfo(
                        on_wait=[waits[-1]], on_update=list(si.on_update)
                    )
                    changed = True
                new.append(inst)
            if changed:
                blk.instructions = new


def _build_layer_nc(tw, mx_sched):
    """One GAT layer, SPMD over 8 cores. tw[w]: tiles per window."""
    nc = bass.Bass(num_devices=NCORES)
    dt = mybir.dt
    twmax = max(tw)

    TAB = nc.dram_tensor("tab", [NPAD, ROWW], dt.bfloat16, kind="ExternalInput")
    WG = nc.dram_tensor("wg", [2, 128, HEADS * C], dt.bfloat16, kind="ExternalInput")
    BIAS = nc.dram_tensor("bias", [1, C], dt.bfloat16, kind="ExternalInput")
    XD = nc.dram_tensor("xd", [NWIN, 128, ROWW], dt.bfloat16, kind="ExternalInput")
    SRC = nc.dram_tensor("src", [128, NWIN * twmax], dt.int32, kind="ExternalInput")
    ADW = nc.dram_tensor("adw", [128, NWIN * 4], dt.bfloat16, kind="ExternalInput")
    EPS = None
    MEDE = nc.dram_tensor(
        "mede", [NWIN, 128, twmax * 256], dt.bfloat16, kind="ExternalInput"
    )
    Y = nc.dram_tensor("y", [NWIN, DW, C], dt.float32, kind="ExternalOutput")

    with tile.TileContext(nc) as tc:
        with (
            tc.tile_pool(name="per", bufs=1) as per,
            tc.tile_pool(name="mw", bufs=4) as mw,
            tc.tile_pool(name="xg", bufs=4) as xgp,
            tc.tile_pool(name="sm", bufs=4) as sm,
            tc.tile_pool(name="mx", bufs=16) as mxp,
            tc.tile_pool(name="zs", bufs=3) as zs,
            tc.tile_pool(name="yt", bufs=3) as yt,
            tc.tile_pool(name="ppz", bufs=2, space="PSUM") as ppz,
            tc.tile_pool(name="pst", bufs=2, space="PSUM") as pst,
            tc.tile_pool(name="pot", bufs=2, space="PSUM") as pot,
        ):
            # ---- resident inputs ----
            wgs = []
            for k in range(2):
                w_ = per.tile([128, HEADS * C], dt.bfloat16, tag=f"wg{k}",
                              name=f"wg{k}")
                nc.scalar.dma_start(w_[:], WG[k])
                wgs.append(w_)
            bia = per.tile([1, C], dt.bfloat16, tag="bias")
            nc.scalar.dma_start(bia[:], BIAS[:])
            ones = per.tile([1, 128], dt.bfloat16, tag="ones")
            nc.vector.memset(ones[:], 1.0)
            epsb = per.tile([1, 4], dt.bfloat16, tag="epsb")
            nc.vector.memset(epsb[:], 1e-30)
            srcm = per.tile([128, NWIN * twmax], dt.int32, tag="srcm", name="srcm")
            nc.sync.dma_start(srcm[:], SRC[:, :])
            adwsb = per.tile([128, NWIN * 4], dt.bfloat16, tag="adw", name="adwsb")
            nc.sync.dma_start(adwsb[:], ADW[:, :])

            for w in range(NWIN):
                twn = tw[w]
                medw = mw.tile([128, twmax * 256], dt.bfloat16, tag="medw",
                               name=f"medw{w}")
                nc.sync.dma_start(medw[:, 0 : twn * 256], MEDE[w, :, 0 : twn * 256])

                # ---- per-tile indirect row gathers (Pool) ----
                xgw = xgp.tile([128, twmax * ROWW], dt.bfloat16, tag="xgw",
                               name=f"xgw{w}")
                sidxw = srcm[:, w * twmax : w * twmax + twn]
                for t in range(twn - 1):
                    nc.gpsimd.indirect_dma_start(
                        out=xgw[:, t * ROWW : (t + 1) * ROWW],
                        out_offset=None,
                        in_=TAB[:, :],
                        in_offset=bass.IndirectOffsetOnAxis(
                            ap=sidxw[:, t : t + 1], axis=0
                        ),
                    )
                # self-loop tile: window dst rows direct from host table
                nc.scalar.dma_start(
                    xgw[:, (twn - 1) * ROWW : twn * ROWW], XD[w]
                )

                # ---- strips: psa (per-edge a_dst), den, recp share one bank ----
                psT = pst.tile([128, 512], dt.float32, tag="pst", name=f"pst{w}")
                for t in range(twn):
                    nc.tensor.matmul(
                        psT[:, 4 * t : 4 * t + 4],
                        lhsT=medw[:, 256 * t + 128 : 256 * t + 256],
                        rhs=adwsb[:, 4 * w : 4 * w + 4],
                        start=True,
                        stop=True,
                    )

                # logits: eff = a_src(gathered) + psa; Prelu; Exp
                eff = sm.tile([128, 4 * twmax], dt.float32, tag="eff",
                              name=f"eff{w}")
                gv = xgw[:, 0 : twn * ROWW].rearrange(
                    "p (t c) -> p t c", t=twn, c=ROWW
                )[:, :, 256:260]
                nc.vector.tensor_add(
                    eff[:, 0 : 4 * twn].rearrange("p (t c) -> p t c", t=twn, c=4),
                    gv,
                    psT[:, 0 : 4 * twn].rearrange("p (t c) -> p t c", t=twn, c=4),
                )
                efl = sm.tile([128, 4 * twmax], dt.float32, tag="efl",
                              name=f"efl{w}")
                nc.scalar.activation(
                    efl[:, 0 : 4 * twn], eff[:, 0 : 4 * twn],
                    mybir.ActivationFunctionType.Prelu, alpha=NEG_SLOPE,
                )
                exwf = sm.tile([128, 4 * twmax], dt.float32, tag="exwf",
                               name=f"exwf{w}")
                nc.scalar.activation(
                    exwf[:, 0 : 4 * twn], efl[:, 0 : 4 * twn],
                    mybir.ActivationFunctionType.Exp,
                )
                exwb = sm.tile([128, 4 * twmax], dt.bfloat16, tag="exwb",
                               name=f"exwb{w}")
                nc.vector.tensor_copy(exwb[:, 0 : 4 * twn], exwf[:, 0 : 4 * twn])

                # den[d,h] = sum_e exw (+eps so unused PSUM rows stay finite)
                nc.tensor.matmul(
                    psT[:, 96:100], lhsT=ones[:], rhs=epsb[:],
                    start=True, stop=False,
                )
                for t in range(twn):
                    nc.tensor.matmul(
                        psT[:, 96:100],
                        lhsT=medw[:, 256 * t : 256 * t + 128],
                        rhs=exwb[:, 4 * t : 4 * t + 4],
                        start=False,
                        stop=(t == twn - 1),
                    )
                recb = sm.tile([128, 4], dt.bfloat16, tag="recb", name=f"recb{w}")
                with nc.allow_low_precision(reason="1/den to bf16 matmul rhs"):
                    nc.vector.reciprocal(recb[:], psT[:, 96:100])
                for t in range(twn):
                    nc.tensor.matmul(
                        psT[:, 128 + 4 * t : 128 + 4 * t + 4],
                        lhsT=medw[:, 256 * t + 128 : 256 * t + 256],
                        rhs=recb[:],
                        start=True,
                        stop=True,
                    )
                exwnf = sm.tile([128, 4 * twmax], dt.float32, tag="exwnf",
                                name=f"exwnf{w}")
                nc.vector.tensor_mul(
                    exwnf[:, 0 : 4 * twn], exwf[:, 0 : 4 * twn],
                    psT[:, 128 : 128 + 4 * twn],
                )

                # ---- weighted aggregation in x-space ----
                zth = [
                    ppz.tile([128, 256], dt.float32, tag=f"zt{ch}",
                             name=f"zt{ch}_{w}")
                    for ch in range(2)
                ]
                for t in range(twn):
                    first = t == 0
                    last = t == twn - 1
                    mx = mxp.tile([128, 256], dt.bfloat16, tag="mx",
                                  name=f"mx_{w}_{t}")
                    if mx_sched[w][t] == 0:
                        # DVE: per-head scalar-ptr multiplies (2x fast path)
                        for h in range(HEADS):
                            nc.vector.tensor_scalar_mul(
                                mx[:, DW * h : DW * (h + 1)],
                                medw[:, 256 * t : 256 * t + DW],
                                exwnf[:, 4 * t + h : 4 * t + h + 1],
                            )
                    else:
                        # Act: per-head scalar-scale copies
                        for h in range(HEADS):
                            nc.scalar.mul(
                                mx[:, DW * h : DW * (h + 1)],
                                medw[:, 256 * t : 256 * t + DW],
                                exwnf[:, 4 * t + h : 4 * t + h + 1],
                            )
                    for ch in range(2):
                        nc.tensor.matmul(
                            zth[ch][:],
                            lhsT=xgw[:, t * ROWW + 128 * ch : t * ROWW + 128 * ch + 128],
                            rhs=mx[:],
                            start=first,
                            stop=last,
                        )

                # ---- per-window transform: out = sum_h z_h @ W_h/4 + b ----
                zsa = zs.tile([128, 512], dt.bfloat16, tag="zsa", name=f"zsa{w}")
                if w % 2 == 0:
                    nc.vector.tensor_copy(zsa[:, 0:256], zth[0][:])
                    nc.scalar.copy(zsa[:, 256:512], zth[1][:])
                else:
                    nc.scalar.copy(zsa[:, 0:256], zth[0][:])
                    nc.vector.tensor_copy(zsa[:, 256:512], zth[1][:])
                outw = pot.tile([DW, C], dt.float32, tag="outw", name=f"outw{w}")
                nc.tensor.matmul(
                    outw[:], lhsT=ones[:, 0:DW], rhs=bia[:], start=True, stop=False,
                )
                for h in range(HEADS):
                    for ch in range(2):
                        nc.tensor.matmul(
                            outw[:],
                            lhsT=zsa[:, 256 * ch + DW * h : 256 * ch + DW * (h + 1)],
                            rhs=wgs[ch][:, C * h : C * (h + 1)],
                            start=False,
                            stop=(h == HEADS - 1 and ch == 1),
                        )
                yacc = yt.tile([DW, C], dt.float32, tag="yacc", name=f"yacc{w}")
                if w % 2 == 0:
                    nc.vector.tensor_copy(yacc[:], outw[:])
                else:
                    nc.scalar.copy(yacc[:], outw[:])
                nc.sync.dma_start(Y[w], yacc[:])

    _split_multiwaits(nc)
    return nc


def _host_prep(edge_index):
    ei = np.asarray(edge_index).astype(np.int64)
    loop = np.arange(N, dtype=np.int64)
    src = np.concatenate([ei[0], loop])
    dst = np.concatenate([ei[1], loop])

    # ---- balance dsts into (core, window) buckets by total degree ----
    NBK = NCORES * NWIN
    deg = np.bincount(dst, minlength=N).astype(np.int64)
    cap = np.array([CAPS[b % NWIN] for b in range(NBK)], np.int64)
    targ = deg.sum() / (N / 128.0)
    targ_b = targ * (cap / 128.0)
    order = np.argsort(-deg, kind="stable")
    L = np.zeros(NBK, np.float64)
    nfill = np.zeros(NBK, np.int64)
    bid = np.zeros(N, np.int64)
    for d in order:
        score = (L + deg[d]) - targ_b
        score[nfill >= cap] = np.inf
        b = int(np.argmin(score))
        L[b] += deg[d]
        bid[d] = b
        nfill[b] += 1

    # swap refinement: drive every bucket load to <= ceil-target so tiles/window
    # hit the minimum (full buckets 17*128 edges, half bucket 9*128-64)
    members = [list(np.where(bid == b)[0]) for b in range(NBK)]
    # total-degree limits so gathered (non-self) tiles hit the minimum count
    limit = np.full(NBK, 8 * 128 + DW, np.float64)
    for _ in range(5000):
        over = L - limit
        b1 = int(np.argmax(over))
        need = over[b1]
        if need <= 0:
            break
        m1 = members[b1]
        deg1 = deg[np.array(m1)]
        best = None
        for b2 in np.argsort(over)[:8]:
            b2 = int(b2)
            if b2 == b1 or over[b2] >= 0:
                continue
            m2 = members[b2]
            deg2 = deg[np.array(m2)]
            room = -over[b2]
            # delta in [1, min(need ceil-slack?, room)]; aim delta ~= need
            dmat = deg1[:, None] - deg2[None, :]
            ok = (dmat > 0) & (dmat <= room)
            if not ok.any():
                continue
            dm = np.where(ok, np.abs(dmat - need), np.inf)
            i, k = np.unravel_index(int(dm.argmin()), dm.shape)
            cand = (float(dm[i, k]), int(dmat[i, k]), b2, int(m1[i]), int(m2[k]))
            if best is None or cand[0] < best[0]:
                best = cand
        if best is None:
            break
        _, delta, b2, d1, d2 = best
        members[b1].remove(d1)
        members[b2].remove(d2)
        members[b1].append(d2)
        members[b2].append(d1)
        bid[d1], bid[d2] = b2, b1
        L[b1] -= delta
        L[b2] += delta

    pj = np.zeros(N, np.int64)
    pw = np.zeros(N, np.int64)
    pslot = np.zeros(N, np.int64)
    for b in range(NBK):
        for s, d in enumerate(members[b]):
            pj[d] = b // NWIN
            pw[d] = b % NWIN
            pslot[d] = s

    # only the E original edges go through the gather path; the N explicit
    # self-loops become one identity tile per window fed by a direct DMA
    nsrc = src[: len(ei[0])]
    ndst = dst[: len(ei[0])]
    ecore_n = pj[ndst]
    ewin_n = pw[ndst]
    dstw_n = pslot[ndst]
    cnt = np.zeros((NCORES, NWIN), np.int64)
    for j in range(NCORES):
        m = ecore_n == j
        for w in range(NWIN):
            cnt[j, w] = int((m & (ewin_n == w)).sum())
    tw = [int(np.ceil(cnt[:, w].max() / 128)) + 1 for w in range(NWIN)]
    twmax = max(tw)
    T = sum(tw)

    srcw = np.full((NCORES, NWIN, 128, twmax), PADROW, np.int32)
    dstwin = np.full((NCORES, NWIN, 128, twmax), -1, np.int64)
    adix = np.full((NCORES, NWIN, 128), PADROW, np.int32)
    dstid = np.arange(N, dtype=np.int64)
    adix[pj, pw, pslot] = dstid
    for j in range(NCORES):
        m = ecore_n == j
        for w in range(NWIN):
            mw_ = m & (ewin_n == w)
            s = nsrc[mw_]
            d = dstw_n[mw_]
            cnte = len(s)
            es = np.arange(cnte)
            srcw[j, w, es % 128, es // 128] = s.astype(np.int32)
            dstwin[j, w, es % 128, es // 128] = d
            # identity self tile over the window's DW dst slots
            dstwin[j, w, 0:DW, tw[w] - 1] = np.arange(DW)

    iota = np.arange(128)
    med = (dstwin[..., None] == iota[None, None, None, None, :]).astype(BF16)
    mde = med.transpose(0, 1, 4, 3, 2).copy()
    mede = np.empty((NCORES, NWIN, 128, twmax, 256), BF16)
    mede[..., 0:128] = med
    mede[..., 128:256] = mde
    mede = mede.reshape(NCORES, NWIN, 128, twmax * 256).copy()

    srcw = srcw.transpose(0, 2, 1, 3).reshape(NCORES, 128, NWIN * twmax).copy()
    return tw, T, srcw, mede, adix, (pj, pw, pslot)


def _fold_attn(W, a):
    """Wa = W @ a per head: [F_in, HEADS]."""
    W64 = np.asarray(W, np.float64)
    A = np.asarray(a, np.float64)
    Wh = W64.reshape(W64.shape[0], HEADS, C)
    return (Wh * A[None]).sum(-1)  # [F_in, HEADS]


def _layer_inputs(x, Wl, a_src, a_dst, bias, adix, placement):
    """Host-side per-layer tensors: TAB, ADW, WG, BIAS, XD."""
    pj, pw, pslot = placement
    x64 = np.asarray(x, np.float64)
    asrc = x64 @ _fold_attn(Wl, a_src)          # [N, 4]
    adst = x64 @ _fold_attn(Wl, a_dst)          # [N, 4]
    tab = np.zeros((NPAD, ROWW), np.float32)
    tab[:N, 0:256] = np.asarray(x, np.float32)
    tab[:N, 256:260] = asrc
    tabb = tab.astype(BF16)
    xd = tabb[adix]                              # [NC, NWIN, 128, ROWW]
    adw = np.zeros((NCORES, NWIN, 128, 4), np.float32)
    adw[pj, pw, pslot] = adst
    adw = adw.transpose(0, 2, 1, 3).reshape(NCORES, 128, NWIN * 4).astype(BF16)
    wg = (np.asarray(Wl, np.float64) * 0.25).astype(BF16).reshape(2, 128, HEADS * C)
    bias_b = np.asarray(bias, np.float32)[None, :].astype(BF16)
    return tabb, adw, wg, bias_b, xd


def _run_layer(nc, tabb, adw, wg, bias_b, xd, srcw, mede, placement):
    in_maps = []
    for j in range(NCORES):
        in_maps.append(
            {
                "tab": tabb,
                "wg": wg,
                "bias": bias_b,
                "xd": xd[j],
                "src": srcw[j],
                "adw": adw[j],
                "mede": mede[j],
            }
        )
    res = run_bass_kernel_spmd(nc, in_maps, core_ids=list(range(NCORES)))
    pj, pw, pslot = placement
    yall = np.stack([res.results[j]["y"] for j in range(NCORES)])  # [NC,NWIN,DW,C]
    y = yall[pj, pw, pslot].astype(np.float32)
    return y


def _mx_schedule(tw):
    """Per (window, tile) engine for the mx broadcast: 0=DVE, 1=Act."""
    return [[0 if (t % 4) != 3 else 1 for t in range(tw[w])] for w in range(NWIN)]


def kernel(kpt_feature, edge_index, W1, a_src1, a_dst1, b1, W2, a_src2, a_dst2, b2):
    key = "k"
    if key not in _cache:
        tw, T, srcw, mede, adix, placement = _host_prep(edge_index)
        nc = _build_layer_nc(tw, _mx_schedule(tw))
        _cache[key] = (nc, tw, T, srcw, mede, adix, placement)
    nc, tw, T, srcw, mede, adix, placement = _cache[key]

    x1 = np.asarray(kpt_feature, np.float32).reshape(N, F)
    y1 = _run_layer(
        nc, *_layer_inputs(x1, W1, a_src1, a_dst1, b1, adix, placement),
        srcw, mede, placement,
    )
    x2 = np.maximum(y1, 0.0)
    y2 = _run_layer(
        nc, *_layer_inputs(x2, W2, a_src2, a_dst2, b2, adix, placement),
        srcw, mede, placement,
    )
    return y2.reshape(B, K, F).astype(np.float32)
